# revision 1
# baseline (speedup 1.0000x reference)
"""Multi-layer bidirectional Tree-LSTM on 8 TRN2 NeuronCores.

Strategy: the input is a complete binary tree of 1024 nodes. Below level 3
there are 8 independent subtrees (rooted at nodes 7..14) -> one subtree per
core (data parallel). The top 7 nodes (0..6) are computed replicated on all
cores; one small AllGather per layer exchanges the 8 subtree-root (h, c)
pairs for the leaves->root direction.

On-device layout is feature-major (hidden dim on partitions, nodes on the
free axis), weights are stationary (bf16, FWL) and node columns stream, so
no transposes are needed anywhere in the recurrence.

Per-core column layout (143 columns):
  0..126   : BFS slots of subtree(7+c)  (slot s, level k=floor(log2(s+1)))
  127      : node 1023 (replicated on every core; only core 0's is used)
  128..134 : top nodes 0..6 (replicated)
  135..142 : subtree roots 7..14 (fwd: from AllGather; bwd: replicated)
"""

import os
import sys

for _p in ("/opt/trn_rl_repo",):
    if _p not in sys.path and os.path.isdir(_p):
        sys.path.insert(0, _p)

import numpy as np
import ml_dtypes

try:
    import jax
    jax.config.update("jax_compilation_cache_dir", os.environ.get("KERNEL_JAX_CACHE", "/tmp/jax_neff_cache"))
    jax.config.update("jax_persistent_cache_min_compile_time_secs", 5.0)
    jax.config.update("jax_persistent_cache_min_entry_size_bytes", 0)
except Exception:
    pass

import concourse.bass as bass
import concourse.mybir as mybir
from concourse import bacc
from concourse.tile import TileContext
from concourse.bass_utils import run_bass_kernel_spmd

BF16 = ml_dtypes.bfloat16
F32 = mybir.dt.float32
B16 = mybir.dt.bfloat16
AF = mybir.ActivationFunctionType

N, D, H, L = 1024, 1024, 512, 2
NCOL = 143  # 127 subtree + node1023 + 7 top + 8 roots
NCORES = 8

_last_results = None  # stashed BassKernelResults for test.py


def _node_ids(c):
    ids = []
    for k in range(7):
        base = (8 + c) * (1 << k) - 1
        ids.extend(range(base, base + (1 << k)))
    ids.append(1023)
    ids.extend(range(0, 7))
    ids.extend(range(7, 15))
    return np.asarray(ids, dtype=np.int64)


def _pack_lhsT(w, kchunks, mchunks):
    # w: [M, K] fp32 -> lhsT tiles [kchunks, mchunks, 128, 128] where
    # tile[k, m, kp, mc] = w[m*128+mc, k*128+kp]
    Mdim, Kdim = w.shape
    assert Mdim == mchunks * 128 and Kdim == kchunks * 128
    t = w.reshape(mchunks, 128, kchunks, 128).transpose(2, 0, 3, 1)
    return np.ascontiguousarray(t.astype(BF16))


def _build_program():
    nc = bacc.Bacc("TRN2", target_bir_lowering=False, debug=False,
                   num_devices=NCORES)

    featsT_d = nc.dram_tensor("featsT", [8, 128, NCOL], B16, kind="ExternalInput")
    wpre_d, wrecf_d, wrecb_d, biasf_d, biasb_d = [], [], [], [], []
    for l in range(L):
        wpre_d.append(nc.dram_tensor(f"wpre{l}", [13, 8, 4, 128, 128], B16,
                                     kind="ExternalInput"))
        wrecf_d.append(nc.dram_tensor(f"wrecf{l}", [8, 24, 128, 128], B16,
                                      kind="ExternalInput"))
        wrecb_d.append(nc.dram_tensor(f"wrecb{l}", [4, 20, 128, 128], B16,
                                      kind="ExternalInput"))
        biasf_d.append(nc.dram_tensor(f"biasf{l}", [128, 28], F32,
                                      kind="ExternalInput"))
        biasb_d.append(nc.dram_tensor(f"biasb{l}", [128, 24], F32,
                                      kind="ExternalInput"))
    mask_d = nc.dram_tensor("mask", [128, 1], F32, kind="ExternalInput")
    psel_d = nc.dram_tensor("psel", [128, 8], F32, kind="ExternalInput")
    out_loc_d = nc.dram_tensor("out_loc", [1024, 128], F32, kind="ExternalOutput")
    out_top_d = nc.dram_tensor("out_top", [1024, 7], F32, kind="ExternalOutput")

    with TileContext(nc) as tc:
        with (
            tc.tile_pool(name="state", bufs=1) as state_p,
            tc.tile_pool(name="weights", bufs=1) as w_p,
            tc.tile_pool(name="pre", bufs=1) as pre_p,
            tc.tile_pool(name="wstream", bufs=2) as ws_p,
            tc.tile_pool(name="scratch", bufs=2) as sc_p,
            tc.tile_pool(name="psum_pre", bufs=2, space="PSUM") as pp_p,
            tc.tile_pool(name="psum_rec", bufs=2, space="PSUM") as pr_p,
            tc.tile_pool(name="dram", bufs=1, space="DRAM") as dram_p,
        ):
            HF = state_p.tile([128, 4, NCOL], F32, name="HF")
            CF = state_p.tile([128, 4, NCOL], F32, name="CF")
            HB = state_p.tile([128, 4, NCOL], F32, name="HB")
            CB = state_p.tile([128, 4, NCOL], F32, name="CB")
            mask_sb = state_p.tile([128, 1], F32, name="mask_sb")
            psel_sb = state_p.tile([128, 8], F32, name="psel_sb")
            nc.sync.dma_start(mask_sb[:], mask_d[:])
            nc.sync.dma_start(psel_sb[:], psel_d[:])

            def fwd_elem(lo, n, ps, lc, rc):
                """gates -> (c, hf) for fwd columns [lo, lo+n)."""
                g = sc_p.tile([128, 24, 65], F32, tag="gates", name="g")
                if ps is None:
                    nc.scalar.activation(g[:, 0:16, :n], PRE_F[:, 0:16, lo:lo + n],
                                         AF.Sigmoid)
                    nc.scalar.activation(g[:, 20:24, :n], PRE_F[:, 20:24, lo:lo + n],
                                         AF.Sigmoid)
                    nc.scalar.activation(g[:, 16:20, :n], PRE_F[:, 16:20, lo:lo + n],
                                         AF.Tanh)
                else:
                    nc.vector.tensor_add(g[:, :, :n], ps[:, 0:24, :n],
                                         PRE_F[:, 0:24, lo:lo + n])
                    nc.scalar.activation(g[:, 0:16, :n], g[:, 0:16, :n], AF.Sigmoid)
                    nc.scalar.activation(g[:, 20:24, :n], g[:, 20:24, :n], AF.Sigmoid)
                    nc.scalar.activation(g[:, 16:20, :n], g[:, 16:20, :n], AF.Tanh)
                cnew = CF[:, :, lo:lo + n]
                t1 = sc_p.tile([128, 4, 65], F32, tag="t1", name="t1")
                t2 = sc_p.tile([128, 4, 65], F32, tag="t2", name="t2")
                # c = ig*u (+ fl*lc + fr*rc)
                nc.vector.tensor_mul(cnew, g[:, 0:4, :n], g[:, 16:20, :n])
                if lc is not None:
                    nc.vector.tensor_mul(t1[:, :, :n], g[:, 8:12, :n], lc)
                    nc.vector.tensor_add(cnew, cnew, t1[:, :, :n])
                    nc.vector.tensor_mul(t2[:, :, :n], g[:, 12:16, :n], rc)
                    nc.vector.tensor_add(cnew, cnew, t2[:, :, :n])
                # hf = og*tanh(c)*r + (1-r)*px = r*(hh - px) + px
                nc.scalar.activation(t1[:, :, :n], cnew, AF.Tanh)
                nc.vector.tensor_mul(t2[:, :, :n], g[:, 4:8, :n], t1[:, :, :n])  # hh
                px = PRE_F[:, 24:28, lo:lo + n]
                nc.vector.tensor_sub(t2[:, :, :n], t2[:, :, :n], px)
                nc.vector.tensor_mul(t2[:, :, :n], g[:, 20:24, :n], t2[:, :, :n])
                nc.vector.tensor_add(HF[:, :, lo:lo + n], t2[:, :, :n], px)

            def bwd_elem(lo, n, ps, pc):
                g = sc_p.tile([128, 24, 65], F32, tag="gates", name="gb")
                if ps is None:
                    nc.scalar.activation(g[:, 0:12, :n], PRE_B[:, 0:12, lo:lo + n],
                                         AF.Sigmoid)
                    nc.scalar.activation(g[:, 16:20, :n], PRE_B[:, 16:20, lo:lo + n],
                                         AF.Sigmoid)
                    nc.scalar.activation(g[:, 12:16, :n], PRE_B[:, 12:16, lo:lo + n],
                                         AF.Tanh)
                else:
                    nc.vector.tensor_add(g[:, 0:20, :n], ps[:, 0:20, :n],
                                         PRE_B[:, 0:20, lo:lo + n])
                    nc.scalar.activation(g[:, 0:12, :n], g[:, 0:12, :n], AF.Sigmoid)
                    nc.scalar.activation(g[:, 16:20, :n], g[:, 16:20, :n], AF.Sigmoid)
                    nc.scalar.activation(g[:, 12:16, :n], g[:, 12:16, :n], AF.Tanh)
                cnew = CB[:, :, lo:lo + n]
                t1 = sc_p.tile([128, 4, 65], F32, tag="t1", name="t1b")
                t2 = sc_p.tile([128, 4, 65], F32, tag="t2", name="t2b")
                nc.vector.tensor_mul(cnew, g[:, 0:4, :n], g[:, 12:16, :n])  # ig*u
                if pc is not None:
                    nc.vector.tensor_mul(t1[:, :, :n], g[:, 8:12, :n], pc)
                    nc.vector.tensor_add(cnew, cnew, t1[:, :, :n])
                nc.scalar.activation(t1[:, :, :n], cnew, AF.Tanh)
                nc.vector.tensor_mul(t2[:, :, :n], g[:, 4:8, :n], t1[:, :, :n])
                px = PRE_B[:, 20:24, lo:lo + n]
                nc.vector.tensor_sub(t2[:, :, :n], t2[:, :, :n], px)
                nc.vector.tensor_mul(t2[:, :, :n], g[:, 16:20, :n], t2[:, :, :n])
                nc.vector.tensor_add(HB[:, :, lo:lo + n], t2[:, :, :n], px)

            def fwd_gemm_step(lo, n, clo, masked=False):
                ch = sc_p.tile([128, 8, 65], B16, tag="ch", name="ch")
                lc = sc_p.tile([128, 4, 65], F32, tag="lc", name="lc")
                rc = sc_p.tile([128, 4, 65], F32, tag="rc", name="rc")
                nc.vector.tensor_copy(ch[:, 0:4, :n], HF[:, :, clo:clo + 2 * n - 1:2])
                nc.vector.tensor_copy(lc[:, :, :n], CF[:, :, clo:clo + 2 * n - 1:2])
                if masked:
                    nc.vector.tensor_scalar_mul(ch[:, 0:4, n - 1:n],
                                                ch[:, 0:4, n - 1:n], mask_sb[:, 0:1])
                    nc.vector.tensor_scalar_mul(lc[:, :, n - 1:n],
                                                lc[:, :, n - 1:n], mask_sb[:, 0:1])
                    nc.vector.tensor_copy(ch[:, 4:8, :n - 1],
                                          HF[:, :, clo + 1:clo + 2 * n - 2:2])
                    nc.vector.memset(ch[:, 4:8, n - 1:n], 0.0)
                    nc.vector.tensor_copy(rc[:, :, :n - 1],
                                          CF[:, :, clo + 1:clo + 2 * n - 2:2])
                    nc.vector.memset(rc[:, :, n - 1:n], 0.0)
                else:
                    nc.vector.tensor_copy(ch[:, 4:8, :n],
                                          HF[:, :, clo + 1:clo + 2 * n:2])
                    nc.vector.tensor_copy(rc[:, :, :n],
                                          CF[:, :, clo + 1:clo + 2 * n:2])
                ps = pr_p.tile([128, 24, 64], F32, tag="rps", name="ps")
                for m in range(24):
                    for k in range(8):
                        nc.tensor.matmul(ps[:, m, :n],
                                         wf_sb[:, (k * 24 + m) * 128:(k * 24 + m + 1) * 128],
                                         ch[:, k, :n],
                                         start=(k == 0), stop=(k == 7))
                fwd_elem(lo, n, ps, lc[:, :, :n], rc[:, :, :n])

            def bwd_gemm_step(lo, n, plo):
                ch = sc_p.tile([128, 8, 65], B16, tag="ch", name="chb")
                pc = sc_p.tile([128, 4, 65], F32, tag="lc", name="pcb")
                if n == 1:
                    nc.vector.tensor_copy(ch[:, 0:4, 0:1], HB[:, :, plo:plo + 1])
                    nc.vector.tensor_copy(pc[:, :, 0:1], CB[:, :, plo:plo + 1])
                else:
                    m2 = n // 2
                    src_h = HB[:, :, plo:plo + m2].unsqueeze(3).broadcast_to(
                        [128, 4, m2, 2])
                    src_c = CB[:, :, plo:plo + m2].unsqueeze(3).broadcast_to(
                        [128, 4, m2, 2])
                    nc.vector.tensor_copy(
                        ch[:, 0:4, 0:n].rearrange("p c (a b) -> p c a b", b=2), src_h)
                    nc.vector.tensor_copy(
                        pc[:, :, 0:n].rearrange("p c (a b) -> p c a b", b=2), src_c)
                ps = pr_p.tile([128, 24, 64], F32, tag="rps", name="psb")
                for m in range(20):
                    for k in range(4):
                        nc.tensor.matmul(ps[:, m, :n],
                                         wb_sb[:, (k * 20 + m) * 128:(k * 20 + m + 1) * 128],
                                         ch[:, k, :n],
                                         start=(k == 0), stop=(k == 3))
                bwd_elem(lo, n, ps, pc[:, :, :n])

            for l in range(L):
                wf_sb = w_p.tile([128, 8 * 24 * 128], B16, tag="wf", name="wf_sb")
                wb_sb = w_p.tile([128, 4 * 20 * 128], B16, tag="wb", name="wb_sb")
                bf_sb = w_p.tile([128, 28], F32, tag="bf", name="bf_sb")
                bb_sb = w_p.tile([128, 24], F32, tag="bb", name="bb_sb")
                wfv = wf_sb[:].rearrange("p (k m c) -> p k m c", k=8, m=24)
                for k in range(8):
                    nc.sync.dma_start(wfv[:, k],
                                      wrecf_d[l][k].rearrange("m p c -> p m c"))
                wbv = wb_sb[:].rearrange("p (k m c) -> p k m c", k=4, m=20)
                for k in range(4):
                    nc.sync.dma_start(wbv[:, k],
                                      wrecb_d[l][k].rearrange("m p c -> p m c"))
                nc.sync.dma_start(bf_sb[:], biasf_d[l][:])
                nc.sync.dma_start(bb_sb[:], biasb_d[l][:])

                PRE_F = pre_p.tile([128, 28, NCOL], F32, tag="pref", name="PRE_F")
                PRE_B = pre_p.tile([128, 24, NCOL], F32, tag="preb", name="PRE_B")

                ftile = pre_p.tile([128, 8, NCOL], B16, tag="ft", name="ftile")
                if l == 0:
                    nc.sync.dma_start(ftile[:],
                                      featsT_d[:].rearrange("k p c -> p k c"))
                else:
                    for k in range(8):
                        src = HF if k < 4 else HB
                        nc.vector.tensor_copy(ftile[:, k, :], src[:, k % 4, :])

                # ---- pre-projections: PRE = W_pre @ feats (feature-major) ----
                for gidx in range(13):
                    wpb = ws_p.tile([128, 8 * 4 * 128], B16, tag="wpre", name="wpb")
                    nc.sync.dma_start(
                        wpb[:].rearrange("p (k mi c) -> p k mi c", k=8, mi=4),
                        wpre_d[l][gidx].rearrange("k mi p c -> p k mi c"))
                    for mi in range(4):
                        m = gidx * 4 + mi
                        ps = pp_p.tile([128, 143], F32, tag="pps", name="pps")
                        for k in range(8):
                            nc.tensor.matmul(
                                ps[:],
                                wpb[:, (k * 4 + mi) * 128:(k * 4 + mi + 1) * 128],
                                ftile[:, k, :],
                                start=(k == 0), stop=(k == 7))
                        if m < 28:
                            nc.scalar.activation(PRE_F[:, m, :], ps[:], AF.Identity,
                                                 bias=bf_sb[:, m:m + 1])
                        else:
                            nc.scalar.activation(PRE_B[:, m - 28, :], ps[:],
                                                 AF.Identity,
                                                 bias=bb_sb[:, m - 28:m - 27])

                # ---- recurrences ----
                # fwd chain (critical path to the AllGather) is emitted first
                # so it gets scheduler priority; the entire bwd chain comes
                # after the AllGather and fills the PE bubble while the
                # collective is in flight.
                fwd_elem(63, 65, None, None, None)  # leaves (slots 63..127)
                # node-511 fix: slot 63 <- left child col 127 (masked), using
                # only the W_l half of wf (k-chunks 0..3). For cores != 0 the
                # mask zeroes the child, making this an idempotent leaf
                # recompute. Must run before the level-8 step below, which
                # consumes slot 63.
                chx = sc_p.tile([128, 8, 65], B16, tag="ch", name="chx")
                lcx = sc_p.tile([128, 4, 65], F32, tag="lc", name="lcx")
                rcx = sc_p.tile([128, 4, 65], F32, tag="rc", name="rcx")
                nc.vector.tensor_copy(chx[:, 0:4, 0:1], HF[:, :, 127:128])
                nc.vector.tensor_scalar_mul(chx[:, 0:4, 0:1], chx[:, 0:4, 0:1],
                                            mask_sb[:, 0:1])
                nc.vector.tensor_copy(lcx[:, :, 0:1], CF[:, :, 127:128])
                nc.vector.tensor_scalar_mul(lcx[:, :, 0:1], lcx[:, :, 0:1],
                                            mask_sb[:, 0:1])
                nc.vector.memset(rcx[:, :, 0:1], 0.0)
                psx = pr_p.tile([128, 24, 64], F32, tag="rps", name="psx")
                for m in range(24):
                    for k in range(4):
                        nc.tensor.matmul(
                            psx[:, m, 0:1],
                            wf_sb[:, (k * 24 + m) * 128:(k * 24 + m + 1) * 128],
                            chx[:, k, 0:1], start=(k == 0), stop=(k == 3))
                fwd_elem(63, 1, psx, lcx[:, :, 0:1], rcx[:, :, 0:1])
                fwd_gemm_step(31, 32, 63)
                fwd_gemm_step(15, 16, 31)
                fwd_gemm_step(7, 8, 15)
                fwd_gemm_step(3, 4, 7)
                fwd_gemm_step(1, 2, 3)
                fwd_gemm_step(0, 1, 1)

                # AllGather the 8 subtree roots' (h, c)
                ccin = dram_p.tile([1024], F32, tag="ccin", name="ccin")
                ccout = dram_p.tile([8, 1024], F32, tag="ccout", name="ccout",
                                    addr_space="Shared")
                nc.sync.dma_start(
                    ccin[0:512].rearrange("(c p) -> p c", c=4, p=128), HF[:, :, 0])
                nc.sync.dma_start(
                    ccin[512:1024].rearrange("(c p) -> p c", c=4, p=128), CF[:, :, 0])
                nc.gpsimd.collective_compute(
                    "AllGather", mybir.AluOpType.bypass,
                    ins=[ccin.opt()], outs=[ccout.opt()],
                    replica_groups=[list(range(NCORES))])
                for ch in range(4):
                    nc.sync.dma_start(
                        HF[:, ch, 135:143],
                        ccout[:, ch * 128:(ch + 1) * 128].rearrange("g p -> p g"))
                    nc.sync.dma_start(
                        CF[:, ch, 135:143],
                        ccout[:, 512 + ch * 128:512 + (ch + 1) * 128].rearrange(
                            "g p -> p g"))

                # bwd chain (independent of the AllGather)
                bwd_elem(128, 1, None, None)        # root node 0
                bwd_gemm_step(129, 2, 128)
                bwd_gemm_step(131, 4, 129)
                bwd_gemm_step(135, 8, 131)
                # copy own root (col 135+c) into local slot 0
                tmp = sc_p.tile([128, 4, 8], F32, tag="pseltmp", name="pseltmp")
                pb = psel_sb[:, :].unsqueeze(1).broadcast_to([128, 4, 8])
                nc.vector.tensor_mul(tmp[:], HB[:, :, 135:143], pb)
                nc.vector.reduce_sum(HB[:, :, 0], tmp[:], mybir.AxisListType.X)
                tmp2 = sc_p.tile([128, 4, 8], F32, tag="pseltmp", name="pseltmp2")
                nc.vector.tensor_mul(tmp2[:], CB[:, :, 135:143], pb)
                nc.vector.reduce_sum(CB[:, :, 0], tmp2[:], mybir.AxisListType.X)
                bwd_gemm_step(1, 2, 0)
                bwd_gemm_step(3, 4, 1)
                bwd_gemm_step(7, 8, 3)
                bwd_gemm_step(15, 16, 7)
                bwd_gemm_step(31, 32, 15)
                bwd_gemm_step(63, 64, 31)
                bwd_gemm_step(127, 1, 63)    # node 1023

                # fwd top levels (consume the AllGather)
                fwd_gemm_step(131, 4, 135)   # top level 2 (nodes 3..6)
                fwd_gemm_step(129, 2, 131)   # top level 1
                fwd_gemm_step(128, 1, 129)   # root

            # ---- outputs ----
            olv = out_loc_d[:].rearrange("(c p) n -> p c n", c=8, p=128)
            nc.sync.dma_start(olv[:, 0:4, :], HF[:, :, 0:128])
            nc.sync.dma_start(olv[:, 4:8, :], HB[:, :, 0:128])
            otv = out_top_d[:].rearrange("(c p) n -> p c n", c=8, p=128)
            nc.sync.dma_start(otv[:, 0:4, :], HF[:, :, 128:135])
            nc.sync.dma_start(otv[:, 4:8, :], HB[:, :, 128:135])

    nc.finalize()
    return nc


_program_cache = None


def kernel(features, f_px_w, f_px_b, f_x_w, f_x_b, f_l_w, f_l_b, f_r_w, f_r_b,
           b_px_w, b_px_b, b_x_w, b_x_b, b_h_w, b_h_b, left, right, parent):
    global _program_cache, _last_results
    features = np.asarray(features, dtype=np.float32)
    as32 = lambda a: np.asarray(a, dtype=np.float32)

    # ---- host-side packing ----
    shared = {}
    for l in range(L):
        wpre = np.concatenate([as32(f_x_w[l]), as32(f_px_w[l]),
                               as32(b_x_w[l]), as32(b_px_w[l])], axis=0)  # [6656,1024]
        t = _pack_lhsT(wpre, 8, 52)                       # [8, 52, 128, 128]
        t = t.reshape(8, 13, 4, 128, 128).transpose(1, 0, 2, 3, 4)
        shared[f"wpre{l}"] = np.ascontiguousarray(t)      # [13, 8, 4, 128, 128]
        wrf = np.concatenate([as32(f_l_w[l]), as32(f_r_w[l])], axis=1)  # [3072,1024]
        shared[f"wrecf{l}"] = _pack_lhsT(wrf, 8, 24)
        shared[f"wrecb{l}"] = _pack_lhsT(as32(b_h_w[l]), 4, 20)
        bf = np.concatenate([as32(f_x_b[l]) + as32(f_l_b[l]) + as32(f_r_b[l]),
                             as32(f_px_b[l])])            # [3584]
        shared[f"biasf{l}"] = np.ascontiguousarray(bf.reshape(28, 128).T)
        bb = np.concatenate([as32(b_x_b[l]) + as32(b_h_b[l]), as32(b_px_b[l])])
        shared[f"biasb{l}"] = np.ascontiguousarray(bb.reshape(24, 128).T)

    in_maps = []
    ids_all = []
    for c in range(NCORES):
        ids = _node_ids(c)
        ids_all.append(ids)
        ft = features[ids].T.astype(BF16)                 # [1024, 143]
        m = {k: v for k, v in shared.items()}
        m["featsT"] = np.ascontiguousarray(ft.reshape(8, 128, NCOL))
        m["mask"] = np.full((128, 1), 1.0 if c == 0 else 0.0, np.float32)
        ps = np.zeros((128, 8), np.float32)
        ps[:, c] = 1.0
        m["psel"] = ps
        in_maps.append(m)

    if _program_cache is None:
        _program_cache = _build_program()
    nc = _program_cache

    trace = bool(os.environ.get("KERNEL_TRACE"))
    tdir = os.environ.get("KERNEL_TRACE_DIR") or None
    res = run_bass_kernel_spmd(nc, in_maps, core_ids=list(range(NCORES)),
                               trace=trace, tmpdir=tdir)
    _last_results = res

    out = np.empty((N, 2 * H), np.float32)
    for c in range(NCORES):
        loc = res.results[c]["out_loc"]                   # [1024, 128]
        nloc = 128 if c == 0 else 127
        out[ids_all[c][0:nloc]] = loc[:, 0:nloc].T
    out[0:7] = res.results[0]["out_top"].T
    return out



# revision 2
# speedup vs baseline: 1.3349x; 1.3349x over previous
"""Multi-layer bidirectional Tree-LSTM on 8 TRN2 NeuronCores.

Strategy: the input is a complete binary tree of 1024 nodes. Below level 3
there are 8 independent subtrees (rooted at nodes 7..14) -> one subtree per
core (data parallel). The top 7 nodes (0..6) are computed replicated on all
cores; one small AllGather per layer exchanges the 8 subtree-root (h, c)
pairs for the leaves->root direction.

On-device layout is feature-major (hidden dim on partitions, nodes on the
free axis), weights are stationary (bf16, FWL) and node columns stream.

Perf structure (v2):
  - weights host-packed to [128, X] contiguous DRAM layouts (one fat
    descriptor per partition), streamed on the Sync queue in consumption
    order; layer-1 weights prefetch during layer 0 (wpre pool 8-deep).
  - fwd and bwd recurrence chains interleaved per level so each level's
    elementwise tail hides under the other chain's matmuls (keeps PE busy,
    HAM clock warm).
  - child gathers on GpSimd; collectives + cc staging DMAs + outputs on
    GpSimd's queue; elementwise on Vector; activations on Scalar.
  - gate blocks reordered host-side to [ig,fl,fr,og,r,u] (fwd) /
    [ig,f,og,r,u] (bwd) so c = sum of products is one fused mul + adds
    and one sigmoid call covers all sigmoid gates.

Per-core column layout (143 columns):
  0..126   : BFS slots of subtree(7+c)  (slot s, level k=floor(log2(s+1)))
  127      : node 1023 (replicated on every core; only core 0's is used)
  128..134 : top nodes 0..6 (replicated)
  135..142 : subtree roots 7..14 (fwd: from AllGather; bwd: replicated)
"""

import os
import sys

for _p in ("/opt/trn_rl_repo",):
    if _p not in sys.path and os.path.isdir(_p):
        sys.path.insert(0, _p)

import numpy as np
import ml_dtypes

try:
    import jax
    jax.config.update("jax_compilation_cache_dir", os.environ.get("KERNEL_JAX_CACHE", "/tmp/jax_neff_cache"))
    jax.config.update("jax_persistent_cache_min_compile_time_secs", 5.0)
    jax.config.update("jax_persistent_cache_min_entry_size_bytes", 0)
except Exception:
    pass

import concourse.bass as bass
import concourse.mybir as mybir
from concourse import bacc
from concourse.tile import TileContext
from concourse.bass_utils import run_bass_kernel_spmd

BF16 = ml_dtypes.bfloat16
F32 = mybir.dt.float32
B16 = mybir.dt.bfloat16
AF = mybir.ActivationFunctionType

N, D, H, L = 1024, 1024, 512, 2
NCOL = 143  # 127 subtree + node1023 + 7 top + 8 roots
NCORES = 8

# gate block permutations (blocks of H rows)
# fwd reference order: ig, og, fl, fr, u, r  ->  new: ig, fl, fr, og, r, u
PERM_F = [0, 2, 3, 1, 5, 4]
# bwd reference order: ig, og, f, u, r       ->  new: ig, f, og, r, u
PERM_B = [0, 2, 1, 4, 3]

_last_results = None  # stashed BassKernelResults for test.py


def _node_ids(c):
    ids = []
    for k in range(7):
        base = (8 + c) * (1 << k) - 1
        ids.extend(range(base, base + (1 << k)))
    ids.append(1023)
    ids.extend(range(0, 7))
    ids.extend(range(7, 15))
    return np.asarray(ids, dtype=np.int64)


def _perm_rows(a, perm):
    return np.concatenate([a[i * H:(i + 1) * H] for i in perm], axis=0)


def _build_program():
    nc = bacc.Bacc("TRN2", target_bir_lowering=False, debug=False,
                   num_devices=NCORES)

    featsT_d = nc.dram_tensor("featsT", [128, 8 * NCOL], B16, kind="ExternalInput")
    wpre_d, wrecf_d, wrecb_d, biasf_d, biasb_d = [], [], [], [], []
    for l in range(L):
        wpre_d.append(nc.dram_tensor(f"wpre{l}", [13, 128, 4096], B16,
                                     kind="ExternalInput"))
        wrecf_d.append(nc.dram_tensor(f"wrecf{l}", [128, 8 * 24 * 128], B16,
                                      kind="ExternalInput"))
        wrecb_d.append(nc.dram_tensor(f"wrecb{l}", [128, 4 * 20 * 128], B16,
                                      kind="ExternalInput"))
        biasf_d.append(nc.dram_tensor(f"biasf{l}", [128, 28], F32,
                                      kind="ExternalInput"))
        biasb_d.append(nc.dram_tensor(f"biasb{l}", [128, 24], F32,
                                      kind="ExternalInput"))
    mask_d = nc.dram_tensor("mask", [128, 1], F32, kind="ExternalInput")
    psel_d = nc.dram_tensor("psel", [128, 8], F32, kind="ExternalInput")
    out_loc_d = nc.dram_tensor("out_loc", [1024, 128], F32, kind="ExternalOutput")
    out_top_d = nc.dram_tensor("out_top", [1024, 7], F32, kind="ExternalOutput")

    with TileContext(nc) as tc:
        with (
            tc.tile_pool(name="state", bufs=1) as state_p,
            tc.tile_pool(name="weights", bufs=1) as w_p,
            tc.tile_pool(name="bias", bufs=2) as b_p,
            tc.tile_pool(name="pre", bufs=1) as pre_p,
            tc.tile_pool(name="wstream", bufs=8) as ws_p,
            tc.tile_pool(name="scratch", bufs=3) as sc_p,
            tc.tile_pool(name="psum_pre", bufs=2, space="PSUM") as pp_p,
            tc.tile_pool(name="psum_rec", bufs=2, space="PSUM") as pr_p,
            tc.tile_pool(name="dram", bufs=1, space="DRAM") as dram_p,
        ):
            HF = state_p.tile([128, 4, NCOL], F32, name="HF")
            CF = state_p.tile([128, 4, NCOL], F32, name="CF")
            HB = state_p.tile([128, 4, NCOL], F32, name="HB")
            CB = state_p.tile([128, 4, NCOL], F32, name="CB")
            mask_sb = state_p.tile([128, 1], F32, name="mask_sb")
            psel_sb = state_p.tile([128, 8], F32, name="psel_sb")
            nc.sync.dma_start(mask_sb[:], mask_d[:])
            nc.sync.dma_start(psel_sb[:], psel_d[:])

            # ---------------- elementwise helpers ----------------
            # fwd gate chunks: 0:4 ig | 4:8 fl | 8:12 fr | 12:16 og
            #                  | 16:20 r | 20:24 u ; PRE_F 24:28 px
            def fwd_elem(lo, n, ps, Bt, nch):
                g = sc_p.tile([128, 24, 65], F32, tag="gates", name="g")
                if ps is None:
                    nc.scalar.activation(g[:, 0:20, :n], PRE_F[:, 0:20, lo:lo + n],
                                         AF.Sigmoid)
                    nc.scalar.activation(Bt[:, 0:4, :n], PRE_F[:, 20:24, lo:lo + n],
                                         AF.Tanh)
                else:
                    nc.vector.tensor_add(g[:, :, :n], ps[:, 0:24, :n],
                                         PRE_F[:, 0:24, lo:lo + n])
                    nc.scalar.activation(g[:, 0:20, :n], g[:, 0:20, :n], AF.Sigmoid)
                    nc.scalar.activation(Bt[:, 0:4, :n], g[:, 20:24, :n], AF.Tanh)
                cn = CF[:, :, lo:lo + n]
                if nch == 2:
                    nc.vector.tensor_mul(g[:, 0:12, :n], g[:, 0:12, :n],
                                         Bt[:, 0:12, :n])
                    nc.vector.tensor_add(cn, g[:, 0:4, :n], g[:, 4:8, :n])
                    nc.vector.tensor_add(cn, cn, g[:, 8:12, :n])
                elif nch == 1:
                    nc.vector.tensor_mul(g[:, 0:8, :n], g[:, 0:8, :n],
                                         Bt[:, 0:8, :n])
                    nc.vector.tensor_add(cn, g[:, 0:4, :n], g[:, 4:8, :n])
                else:
                    nc.vector.tensor_mul(cn, g[:, 0:4, :n], Bt[:, 0:4, :n])
                t1 = sc_p.tile([128, 4, 65], F32, tag="t1", name="t1")
                t2 = sc_p.tile([128, 4, 65], F32, tag="t2", name="t2")
                nc.scalar.activation(t1[:, :, :n], cn, AF.Tanh)
                nc.vector.tensor_mul(t2[:, :, :n], g[:, 12:16, :n], t1[:, :, :n])
                px = PRE_F[:, 24:28, lo:lo + n]
                nc.vector.tensor_sub(t2[:, :, :n], t2[:, :, :n], px)
                nc.vector.tensor_mul(t2[:, :, :n], g[:, 16:20, :n], t2[:, :, :n])
                nc.vector.tensor_add(HF[:, :, lo:lo + n], t2[:, :, :n], px)

            # bwd gate chunks: 0:4 ig | 4:8 f | 8:12 og | 12:16 r | 16:20 u
            # PRE_B 20:24 px
            def bwd_elem(lo, n, ps, Bt):
                g = sc_p.tile([128, 24, 65], F32, tag="gates", name="gb")
                if ps is None:
                    nc.scalar.activation(g[:, 0:16, :n], PRE_B[:, 0:16, lo:lo + n],
                                         AF.Sigmoid)
                    nc.scalar.activation(Bt[:, 0:4, :n], PRE_B[:, 16:20, lo:lo + n],
                                         AF.Tanh)
                else:
                    nc.vector.tensor_add(g[:, 0:20, :n], ps[:, 0:20, :n],
                                         PRE_B[:, 0:20, lo:lo + n])
                    nc.scalar.activation(g[:, 0:16, :n], g[:, 0:16, :n], AF.Sigmoid)
                    nc.scalar.activation(Bt[:, 0:4, :n], g[:, 16:20, :n], AF.Tanh)
                cn = CB[:, :, lo:lo + n]
                if ps is None:
                    nc.vector.tensor_mul(cn, g[:, 0:4, :n], Bt[:, 0:4, :n])
                else:
                    nc.vector.tensor_mul(g[:, 0:8, :n], g[:, 0:8, :n],
                                         Bt[:, 0:8, :n])
                    nc.vector.tensor_add(cn, g[:, 0:4, :n], g[:, 4:8, :n])
                t1 = sc_p.tile([128, 4, 65], F32, tag="t1", name="t1b")
                t2 = sc_p.tile([128, 4, 65], F32, tag="t2", name="t2b")
                nc.scalar.activation(t1[:, :, :n], cn, AF.Tanh)
                nc.vector.tensor_mul(t2[:, :, :n], g[:, 8:12, :n], t1[:, :, :n])
                px = PRE_B[:, 20:24, lo:lo + n]
                nc.vector.tensor_sub(t2[:, :, :n], t2[:, :, :n], px)
                nc.vector.tensor_mul(t2[:, :, :n], g[:, 12:16, :n], t2[:, :, :n])
                nc.vector.tensor_add(HB[:, :, lo:lo + n], t2[:, :, :n], px)

            # ---------------- recurrence units ----------------
            def fwd_unit(lo, n, clo):
                Bt = sc_p.tile([128, 12, 65], F32, tag="B", name="Bf")
                ch = sc_p.tile([128, 8, 65], B16, tag="ch", name="ch")
                nc.gpsimd.tensor_copy(ch[:, 0:4, :n],
                                      HF[:, :, clo:clo + 2 * n - 1:2])
                nc.gpsimd.tensor_copy(ch[:, 4:8, :n],
                                      HF[:, :, clo + 1:clo + 2 * n:2])
                nc.gpsimd.tensor_copy(Bt[:, 4:8, :n],
                                      CF[:, :, clo:clo + 2 * n - 1:2])
                nc.gpsimd.tensor_copy(Bt[:, 8:12, :n],
                                      CF[:, :, clo + 1:clo + 2 * n:2])
                ps = pr_p.tile([128, 24, 64], F32, tag="rps", name="ps")
                for m in range(24):
                    for k in range(8):
                        nc.tensor.matmul(
                            ps[:, m, :n],
                            wf_sb[:, (k * 24 + m) * 128:(k * 24 + m + 1) * 128],
                            ch[:, k, :n], start=(k == 0), stop=(k == 7))
                fwd_elem(lo, n, ps, Bt, 2)

            def fwd_fix511():
                # node-511 fix: slot 63 <- left child col 127 (masked). For
                # cores != 0 the mask zeroes the child, making this an
                # idempotent leaf recompute. Uses only the W_l half (k 0..3).
                Bt = sc_p.tile([128, 12, 65], F32, tag="B", name="Bx")
                ch = sc_p.tile([128, 8, 65], B16, tag="ch", name="chx")
                nc.gpsimd.tensor_copy(ch[:, 0:4, 0:1], HF[:, :, 127:128])
                nc.gpsimd.tensor_scalar_mul(ch[:, 0:4, 0:1], ch[:, 0:4, 0:1],
                                            mask_sb[:, 0:1])
                nc.gpsimd.tensor_copy(Bt[:, 4:8, 0:1], CF[:, :, 127:128])
                nc.gpsimd.tensor_scalar_mul(Bt[:, 4:8, 0:1], Bt[:, 4:8, 0:1],
                                            mask_sb[:, 0:1])
                ps = pr_p.tile([128, 24, 64], F32, tag="rps", name="psx")
                for m in range(24):
                    for k in range(4):
                        nc.tensor.matmul(
                            ps[:, m, 0:1],
                            wf_sb[:, (k * 24 + m) * 128:(k * 24 + m + 1) * 128],
                            ch[:, k, 0:1], start=(k == 0), stop=(k == 3))
                fwd_elem(63, 1, ps, Bt, 1)

            def bwd_unit(lo, n, plo):
                Bt = sc_p.tile([128, 12, 65], F32, tag="B", name="Bb")
                ch = sc_p.tile([128, 8, 65], B16, tag="ch", name="chb")
                if n == 1:
                    nc.gpsimd.tensor_copy(ch[:, 0:4, 0:1], HB[:, :, plo:plo + 1])
                    nc.gpsimd.tensor_copy(Bt[:, 4:8, 0:1], CB[:, :, plo:plo + 1])
                else:
                    m2 = n // 2
                    src_h = HB[:, :, plo:plo + m2].unsqueeze(3).broadcast_to(
                        [128, 4, m2, 2])
                    src_c = CB[:, :, plo:plo + m2].unsqueeze(3).broadcast_to(
                        [128, 4, m2, 2])
                    nc.gpsimd.tensor_copy(
                        ch[:, 0:4, 0:n].rearrange("p c (a b) -> p c a b", b=2),
                        src_h)
                    nc.gpsimd.tensor_copy(
                        Bt[:, 4:8, 0:n].rearrange("p c (a b) -> p c a b", b=2),
                        src_c)
                ps = pr_p.tile([128, 24, 64], F32, tag="rps", name="psb")
                for m in range(20):
                    for k in range(4):
                        nc.tensor.matmul(
                            ps[:, m, :n],
                            wb_sb[:, (k * 20 + m) * 128:(k * 20 + m + 1) * 128],
                            ch[:, k, :n], start=(k == 0), stop=(k == 3))
                bwd_elem(lo, n, ps, Bt)

            def bwd_root():
                Bt = sc_p.tile([128, 12, 65], F32, tag="B", name="Br")
                bwd_elem(128, 1, None, Bt)

            def bwd_sel():
                # copy own root (col 135+c) into local slot 0
                tmp = sc_p.tile([128, 4, 8], F32, tag="pseltmp", name="pseltmp")
                pb = psel_sb[:, :].unsqueeze(1).broadcast_to([128, 4, 8])
                nc.vector.tensor_mul(tmp[:], HB[:, :, 135:143], pb)
                nc.vector.reduce_sum(HB[:, :, 0], tmp[:], mybir.AxisListType.X)
                tmp2 = sc_p.tile([128, 4, 8], F32, tag="pseltmp", name="pseltmp2")
                nc.vector.tensor_mul(tmp2[:], CB[:, :, 135:143], pb)
                nc.vector.reduce_sum(CB[:, :, 0], tmp2[:], mybir.AxisListType.X)

            def pre_chunk(gidx, wpb):
                for mi in range(4):
                    m = gidx * 4 + mi
                    ps = pp_p.tile([128, 143], F32, tag="pps", name="pps")
                    for k in range(8):
                        nc.tensor.matmul(
                            ps[:],
                            wpb[:, (k * 4 + mi) * 128:(k * 4 + mi + 1) * 128],
                            ftile[:, k, :], start=(k == 0), stop=(k == 7))
                    if m < 28:
                        nc.scalar.activation(PRE_F[:, m, :], ps[:], AF.Identity,
                                             bias=bf_sb[:, m:m + 1])
                    else:
                        nc.scalar.activation(PRE_B[:, m - 28, :], ps[:],
                                             AF.Identity,
                                             bias=bb_sb[:, m - 28:m - 27])

            # ---------------- layer loop ----------------
            for l in range(L):
                # weight stream block (sync queue) in consumption order;
                # for l=1 these triggers sit behind l=0's and prefetch into
                # free slots during layer-0 compute.
                bf_sb = b_p.tile([128, 28], F32, tag="bf", name="bf_sb")
                bb_sb = b_p.tile([128, 24], F32, tag="bb", name="bb_sb")
                nc.sync.dma_start(bf_sb[:], biasf_d[l][:])
                nc.sync.dma_start(bb_sb[:], biasb_d[l][:])
                wpb_list = []
                for g in range(13):
                    wpb = ws_p.tile([128, 4096], B16, tag="wpre", name=f"wpb{g}")
                    nc.sync.dma_start(wpb[:], wpre_d[l][g])
                    wpb_list.append(wpb)
                wf_sb = w_p.tile([128, 8 * 24 * 128], B16, tag="wf", name="wf_sb")
                wb_sb = w_p.tile([128, 4 * 20 * 128], B16, tag="wb", name="wb_sb")
                nc.sync.dma_start(wf_sb[:], wrecf_d[l][:])
                nc.sync.dma_start(wb_sb[:], wrecb_d[l][:])

                PRE_F = pre_p.tile([128, 28, NCOL], B16, tag="pref", name="PRE_F")
                PRE_B = pre_p.tile([128, 24, NCOL], B16, tag="preb", name="PRE_B")

                if l == 0:
                    ftile = pre_p.tile([128, 8, NCOL], B16, tag="ft", name="ftile")
                    nc.sync.dma_start(
                        ftile[:].rearrange("p c n -> p (c n)"), featsT_d[:])

                # ---- pre-projections: PRE = W_pre @ feats (feature-major) ----
                for g in range(7):          # PRE_F chunks (m 0..27)
                    pre_chunk(g, wpb_list[g])
                fwd_elem(63, 65, None,
                         sc_p.tile([128, 12, 65], F32, tag="B", name="Bleaf"), 0)
                for g in range(7, 13):      # PRE_B chunks (m 28..51)
                    pre_chunk(g, wpb_list[g])
                bwd_root()
                fwd_fix511()

                # ---- interleaved recurrence ----
                fwd_unit(31, 32, 63)
                bwd_unit(129, 2, 128)
                fwd_unit(15, 16, 31)
                bwd_unit(131, 4, 129)
                fwd_unit(7, 8, 15)
                bwd_unit(135, 8, 131)
                fwd_unit(3, 4, 7)
                bwd_sel()
                bwd_unit(1, 2, 0)
                fwd_unit(1, 2, 3)
                bwd_unit(3, 4, 1)
                fwd_unit(0, 1, 1)

                # AllGather the 8 subtree roots' (h, c) — staged on gpsimd
                ccin = dram_p.tile([1024], F32, tag="ccin", name="ccin")
                ccout = dram_p.tile([8, 1024], F32, tag="ccout", name="ccout",
                                    addr_space="Shared")
                nc.gpsimd.dma_start(
                    ccin[0:512].rearrange("(c p) -> p c", c=4, p=128), HF[:, :, 0])
                nc.gpsimd.dma_start(
                    ccin[512:1024].rearrange("(c p) -> p c", c=4, p=128), CF[:, :, 0])
                nc.gpsimd.collective_compute(
                    "AllGather", mybir.AluOpType.bypass,
                    ins=[ccin.opt()], outs=[ccout.opt()],
                    replica_groups=[list(range(NCORES))])

                # bwd chain fills the AllGather window
                bwd_unit(7, 8, 3)
                bwd_unit(15, 16, 7)
                bwd_unit(31, 32, 15)
                bwd_unit(63, 64, 31)
                bwd_unit(127, 1, 63)    # node 1023

                # AG consumers (gpsimd queue; they wait on the collective)
                for cc in range(4):
                    nc.gpsimd.dma_start(
                        HF[:, cc, 135:143],
                        ccout[:, cc * 128:(cc + 1) * 128].rearrange("g p -> p g"))
                    nc.gpsimd.dma_start(
                        CF[:, cc, 135:143],
                        ccout[:, 512 + cc * 128:512 + (cc + 1) * 128].rearrange(
                            "g p -> p g"))

                # fwd top levels (consume the AllGather)
                fwd_unit(131, 4, 135)   # top level 2 (nodes 3..6)
                fwd_unit(129, 2, 131)   # top level 1
                fwd_unit(128, 1, 129)   # root

                if l + 1 < L:
                    ftile = pre_p.tile([128, 8, NCOL], B16, tag="ft",
                                       name="ftile1")
                    for k in range(8):
                        src = HF if k < 4 else HB
                        nc.vector.tensor_copy(ftile[:, k, :], src[:, k % 4, :])

            # ---- outputs ----
            olv = out_loc_d[:].rearrange("(c p) n -> p c n", c=8, p=128)
            nc.gpsimd.dma_start(olv[:, 0:4, :], HF[:, :, 0:128])
            nc.gpsimd.dma_start(olv[:, 4:8, :], HB[:, :, 0:128])
            otv = out_top_d[:].rearrange("(c p) n -> p c n", c=8, p=128)
            nc.gpsimd.dma_start(otv[:, 0:4, :], HF[:, :, 128:135])
            nc.gpsimd.dma_start(otv[:, 4:8, :], HB[:, :, 128:135])

    nc.finalize()
    return nc


_program_cache = None


def kernel(features, f_px_w, f_px_b, f_x_w, f_x_b, f_l_w, f_l_b, f_r_w, f_r_b,
           b_px_w, b_px_b, b_x_w, b_x_b, b_h_w, b_h_b, left, right, parent):
    global _program_cache, _last_results
    features = np.asarray(features, dtype=np.float32)
    as32 = lambda a: np.asarray(a, dtype=np.float32)

    # ---- host-side packing ----
    shared = {}
    for l in range(L):
        fx = _perm_rows(as32(f_x_w[l]), PERM_F)
        bx = _perm_rows(as32(b_x_w[l]), PERM_B)
        wpre = np.concatenate([fx, as32(f_px_w[l]), bx, as32(b_px_w[l])],
                              axis=0)                      # [6656, 1024]
        t = wpre.reshape(13, 4, 128, 8, 128).transpose(0, 4, 3, 1, 2)
        shared[f"wpre{l}"] = np.ascontiguousarray(
            t.reshape(13, 128, 4096).astype(BF16))

        wrf = np.concatenate([_perm_rows(as32(f_l_w[l]), PERM_F),
                              _perm_rows(as32(f_r_w[l]), PERM_F)],
                             axis=1)                       # [3072, 1024]
        t = wrf.reshape(24, 128, 8, 128).transpose(3, 2, 0, 1)
        shared[f"wrecf{l}"] = np.ascontiguousarray(
            t.reshape(128, 8 * 24 * 128).astype(BF16))

        wrb = _perm_rows(as32(b_h_w[l]), PERM_B)           # [2560, 512]
        t = wrb.reshape(20, 128, 4, 128).transpose(3, 2, 0, 1)
        shared[f"wrecb{l}"] = np.ascontiguousarray(
            t.reshape(128, 4 * 20 * 128).astype(BF16))

        bf = np.concatenate([
            _perm_rows(as32(f_x_b[l]) + as32(f_l_b[l]) + as32(f_r_b[l]), PERM_F),
            as32(f_px_b[l])])                              # [3584]
        shared[f"biasf{l}"] = np.ascontiguousarray(bf.reshape(28, 128).T)
        bb = np.concatenate([
            _perm_rows(as32(b_x_b[l]) + as32(b_h_b[l]), PERM_B),
            as32(b_px_b[l])])
        shared[f"biasb{l}"] = np.ascontiguousarray(bb.reshape(24, 128).T)

    in_maps = []
    ids_all = []
    for c in range(NCORES):
        ids = _node_ids(c)
        ids_all.append(ids)
        ft = features[ids]                                 # [143, 1024]
        ftT = ft.T.reshape(8, 128, NCOL).transpose(1, 0, 2)  # [128, 8, 143]
        m = {k: v for k, v in shared.items()}
        m["featsT"] = np.ascontiguousarray(
            ftT.reshape(128, 8 * NCOL).astype(BF16))
        m["mask"] = np.full((128, 1), 1.0 if c == 0 else 0.0, np.float32)
        ps = np.zeros((128, 8), np.float32)
        ps[:, c] = 1.0
        m["psel"] = ps
        in_maps.append(m)

    if _program_cache is None:
        _program_cache = _build_program()
    nc = _program_cache

    trace = bool(os.environ.get("KERNEL_TRACE"))
    tdir = os.environ.get("KERNEL_TRACE_DIR") or None
    res = run_bass_kernel_spmd(nc, in_maps, core_ids=list(range(NCORES)),
                               trace=trace, tmpdir=tdir)
    _last_results = res

    out = np.empty((N, 2 * H), np.float32)
    for c in range(NCORES):
        loc = res.results[c]["out_loc"]                    # [1024, 128]
        nloc = 128 if c == 0 else 127
        out[ids_all[c][0:nloc]] = loc[:, 0:nloc].T
    out[0:7] = res.results[0]["out_top"].T
    return out


# revision 12
# speedup vs baseline: 1.3537x; 1.0141x over previous
"""Multi-layer bidirectional Tree-LSTM on 8 TRN2 NeuronCores.

Strategy: the input is a complete binary tree of 1024 nodes. Below level 3
there are 8 independent subtrees (rooted at nodes 7..14) -> one subtree per
core (data parallel). The top 7 nodes (0..6) are computed replicated on all
cores; one small AllGather per layer exchanges the 8 subtree-root (h, c)
pairs for the leaves->root direction.

On-device layout is feature-major (hidden dim on partitions, nodes on the
free axis), weights are stationary (bf16, FWL) and node columns stream.

Perf structure (v2):
  - weights host-packed to [128, X] contiguous DRAM layouts (one fat
    descriptor per partition), streamed on the Sync queue in consumption
    order; layer-1 weights prefetch during layer 0 (wpre pool 8-deep).
  - fwd and bwd recurrence chains interleaved per level so each level's
    elementwise tail hides under the other chain's matmuls (keeps PE busy,
    HAM clock warm).
  - child gathers on GpSimd; collectives + cc staging DMAs + outputs on
    GpSimd's queue; elementwise on Vector; activations on Scalar.
  - gate blocks reordered host-side to [ig,fl,fr,og,r,u] (fwd) /
    [ig,f,og,r,u] (bwd) so c = sum of products is one fused mul + adds
    and one sigmoid call covers all sigmoid gates.

Per-core column layout (143 columns):
  0..126   : BFS slots of subtree(7+c)  (slot s, level k=floor(log2(s+1)))
  127      : node 1023 (replicated on every core; only core 0's is used)
  128..134 : top nodes 0..6 (replicated)
  135..142 : subtree roots 7..14 (fwd: from AllGather; bwd: replicated)
"""

import os
import sys

for _p in ("/opt/trn_rl_repo",):
    if _p not in sys.path and os.path.isdir(_p):
        sys.path.insert(0, _p)

import numpy as np
import ml_dtypes

try:
    import jax
    jax.config.update("jax_compilation_cache_dir", os.environ.get("KERNEL_JAX_CACHE", "/tmp/jax_neff_cache"))
    jax.config.update("jax_persistent_cache_min_compile_time_secs", 5.0)
    jax.config.update("jax_persistent_cache_min_entry_size_bytes", 0)
except Exception:
    pass

import concourse.bass as bass
import concourse.mybir as mybir
from concourse import bacc
from concourse.tile import TileContext
from concourse.bass_utils import run_bass_kernel_spmd

BF16 = ml_dtypes.bfloat16
F32 = mybir.dt.float32
B16 = mybir.dt.bfloat16
AF = mybir.ActivationFunctionType

N, D, H, L = 1024, 1024, 512, 2
NCOL = 143  # 127 subtree + node1023 + 7 top + 8 roots
NCORES = 8

# gate block permutations (blocks of H rows)
# fwd reference order: ig, og, fl, fr, u, r  ->  new: ig, fl, fr, u, og, r
# (c-path gates first so the first PSUM group [ig,fl,fr,u] unblocks the
#  c computation while the [og,r] group is still in the matmul queue)
PERM_F = [0, 2, 3, 4, 1, 5]
# bwd reference order: ig, og, f, u, r       ->  new: ig, f, u, og, r
PERM_B = [0, 2, 3, 1, 4]

_last_results = None  # stashed BassKernelResults for test.py


def _node_ids(c):
    ids = []
    for k in range(7):
        base = (8 + c) * (1 << k) - 1
        ids.extend(range(base, base + (1 << k)))
    ids.append(1023)
    ids.extend(range(0, 7))
    ids.extend(range(7, 15))
    return np.asarray(ids, dtype=np.int64)


def _perm_rows(a, perm):
    return np.concatenate([a[i * H:(i + 1) * H] for i in perm], axis=0)


def _build_program():
    nc = bacc.Bacc("TRN2", target_bir_lowering=False, debug=False,
                   num_devices=NCORES)

    featsT_d = nc.dram_tensor("featsT", [128, 8 * NCOL], B16, kind="ExternalInput")
    wpre_d, wrecf_d, wrecb_d, biasf_d, biasb_d = [], [], [], [], []
    for l in range(L):
        wpre_d.append(nc.dram_tensor(f"wpre{l}", [13, 128, 4096], B16,
                                     kind="ExternalInput"))
        wrecf_d.append(nc.dram_tensor(f"wrecf{l}", [128, 8 * 24 * 128], B16,
                                      kind="ExternalInput"))
        wrecb_d.append(nc.dram_tensor(f"wrecb{l}", [128, 4 * 20 * 128], B16,
                                      kind="ExternalInput"))
        biasf_d.append(nc.dram_tensor(f"biasf{l}", [128, 28], F32,
                                      kind="ExternalInput"))
        biasb_d.append(nc.dram_tensor(f"biasb{l}", [128, 24], F32,
                                      kind="ExternalInput"))
    mask_d = nc.dram_tensor("mask", [128, 1], F32, kind="ExternalInput")
    psel_d = nc.dram_tensor("psel", [128, 8], F32, kind="ExternalInput")
    out_loc_d = nc.dram_tensor("out_loc", [1024, 128], F32, kind="ExternalOutput")
    out_top_d = nc.dram_tensor("out_top", [1024, 7], F32, kind="ExternalOutput")

    with TileContext(nc) as tc:
        with (
            tc.tile_pool(name="state", bufs=1) as state_p,
            tc.tile_pool(name="weights", bufs=1) as w_p,
            tc.tile_pool(name="bias", bufs=2) as b_p,
            tc.tile_pool(name="pre", bufs=1) as pre_p,
            tc.tile_pool(name="wstream", bufs=8) as ws_p,
            tc.tile_pool(name="scratch", bufs=3) as sc_p,
            tc.tile_pool(name="psum_pre", bufs=2, space="PSUM") as pp_p,
            tc.tile_pool(name="psum_reca", bufs=2, space="PSUM") as pra_p,
            tc.tile_pool(name="psum_recb", bufs=2, space="PSUM") as prb_p,
            tc.tile_pool(name="dram", bufs=1, space="DRAM") as dram_p,
        ):
            HF = state_p.tile([128, 4, NCOL], F32, name="HF")
            CF = state_p.tile([128, 4, NCOL], F32, name="CF")
            HB = state_p.tile([128, 4, NCOL], F32, name="HB")
            CB = state_p.tile([128, 4, NCOL], F32, name="CB")
            mask_sb = state_p.tile([128, 1], F32, name="mask_sb")
            psel_sb = state_p.tile([128, 8], F32, name="psel_sb")
            nc.scalar.dma_start(mask_sb[:], mask_d[:])
            nc.scalar.dma_start(psel_sb[:], psel_d[:])

            # ---------------- elementwise helpers ----------------
            # fwd gate chunks: 0:4 ig | 4:8 fl | 8:12 fr | 12:16 u
            #                  | 16:20 og | 20:24 r ; PRE_F 24:28 px
            # psum group a = chunks 0..15 (c path), group b = 16..23 (h path)
            def fwd_elem_a(lo, n, psa, Bt, g, nch):
                cn = CF[:, :, lo:lo + n]
                if psa is None:
                    nc.scalar.activation(g[:, 0:12, :n], PRE_F[:, 0:12, lo:lo + n],
                                         AF.Sigmoid)
                    nc.scalar.activation(Bt[:, 0:4, :n], PRE_F[:, 12:16, lo:lo + n],
                                         AF.Tanh)
                else:
                    nc.vector.tensor_add(g[:, 0:16, :n], psa[:, 0:16, :n],
                                         PRE_F[:, 0:16, lo:lo + n])
                    nc.scalar.activation(g[:, 0:12, :n], g[:, 0:12, :n], AF.Sigmoid)
                    nc.scalar.activation(Bt[:, 0:4, :n], g[:, 12:16, :n], AF.Tanh)
                if nch == 2:
                    nc.vector.tensor_mul(g[:, 0:12, :n], g[:, 0:12, :n],
                                         Bt[:, 0:12, :n])
                    nc.vector.tensor_add(cn, g[:, 0:4, :n], g[:, 4:8, :n])
                    nc.vector.tensor_add(cn, cn, g[:, 8:12, :n])
                elif nch == 1:
                    nc.vector.tensor_mul(g[:, 0:8, :n], g[:, 0:8, :n],
                                         Bt[:, 0:8, :n])
                    nc.vector.tensor_add(cn, g[:, 0:4, :n], g[:, 4:8, :n])
                else:
                    nc.vector.tensor_mul(cn, g[:, 0:4, :n], Bt[:, 0:4, :n])

            def fwd_elem_b(lo, n, psb, g, t1):
                cn = CF[:, :, lo:lo + n]
                nc.scalar.activation(t1[:, :, :n], cn, AF.Tanh)
                if psb is None:
                    nc.scalar.activation(g[:, 16:24, :n], PRE_F[:, 16:24, lo:lo + n],
                                         AF.Sigmoid)
                else:
                    nc.vector.tensor_add(g[:, 16:24, :n], psb[:, 0:8, :n],
                                         PRE_F[:, 16:24, lo:lo + n])
                    nc.scalar.activation(g[:, 16:24, :n], g[:, 16:24, :n],
                                         AF.Sigmoid)
                t2 = sc_p.tile([128, 4, 65], F32, tag="t2", name="t2")
                nc.vector.tensor_mul(t2[:, :, :n], g[:, 16:20, :n], t1[:, :, :n])
                px = PRE_F[:, 24:28, lo:lo + n]
                nc.vector.tensor_sub(t2[:, :, :n], t2[:, :, :n], px)
                nc.vector.tensor_mul(t2[:, :, :n], g[:, 20:24, :n], t2[:, :, :n])
                nc.vector.tensor_add(HF[:, :, lo:lo + n], t2[:, :, :n], px)

            # bwd gate chunks: 0:4 ig | 4:8 f | 8:12 u | 12:16 og | 16:20 r
            # PRE_B 20:24 px ; psum group a = 0..11, group b = 12..19
            def bwd_elem_a(lo, n, psa, Bt, g):
                cn = CB[:, :, lo:lo + n]
                if psa is None:
                    nc.scalar.activation(g[:, 0:8, :n], PRE_B[:, 0:8, lo:lo + n],
                                         AF.Sigmoid)
                    nc.scalar.activation(Bt[:, 0:4, :n], PRE_B[:, 8:12, lo:lo + n],
                                         AF.Tanh)
                    nc.vector.tensor_mul(cn, g[:, 0:4, :n], Bt[:, 0:4, :n])
                else:
                    nc.vector.tensor_add(g[:, 0:12, :n], psa[:, 0:12, :n],
                                         PRE_B[:, 0:12, lo:lo + n])
                    nc.scalar.activation(g[:, 0:8, :n], g[:, 0:8, :n], AF.Sigmoid)
                    nc.scalar.activation(Bt[:, 0:4, :n], g[:, 8:12, :n], AF.Tanh)
                    nc.vector.tensor_mul(g[:, 0:8, :n], g[:, 0:8, :n],
                                         Bt[:, 0:8, :n])
                    nc.vector.tensor_add(cn, g[:, 0:4, :n], g[:, 4:8, :n])

            def bwd_elem_b(lo, n, psb, g, t1):
                cn = CB[:, :, lo:lo + n]
                nc.scalar.activation(t1[:, :, :n], cn, AF.Tanh)
                if psb is None:
                    nc.scalar.activation(g[:, 12:20, :n], PRE_B[:, 12:20, lo:lo + n],
                                         AF.Sigmoid)
                else:
                    nc.vector.tensor_add(g[:, 12:20, :n], psb[:, 0:8, :n],
                                         PRE_B[:, 12:20, lo:lo + n])
                    nc.scalar.activation(g[:, 12:20, :n], g[:, 12:20, :n],
                                         AF.Sigmoid)
                t2 = sc_p.tile([128, 4, 65], F32, tag="t2", name="t2b")
                nc.vector.tensor_mul(t2[:, :, :n], g[:, 12:16, :n], t1[:, :, :n])
                px = PRE_B[:, 20:24, lo:lo + n]
                nc.vector.tensor_sub(t2[:, :, :n], t2[:, :, :n], px)
                nc.vector.tensor_mul(t2[:, :, :n], g[:, 16:20, :n], t2[:, :, :n])
                nc.vector.tensor_add(HB[:, :, lo:lo + n], t2[:, :, :n], px)

            # ---------------- recurrence units ----------------
            def fwd_unit(lo, n, clo):
                Bt = sc_p.tile([128, 12, 65], F32, tag="B", name="Bf")
                ch = sc_p.tile([128, 8, 65], B16, tag="ch", name="ch")
                g = sc_p.tile([128, 24, 65], F32, tag="gates", name="g")
                t1 = sc_p.tile([128, 4, 65], F32, tag="t1", name="t1")
                # child h gather (MM-critical) on Scalar, one fused op
                nc.scalar.activation(
                    ch[:].rearrange("p (s c) n -> p s c n", s=2)[:, :, :, :n],
                    HF[:, :, clo:clo + 2 * n].rearrange(
                        "p c (n s) -> p s c n", s=2),
                    AF.Identity)
                # child c gather (elem-critical) on GpSimd
                nc.gpsimd.tensor_copy(
                    Bt[:, 4:12, :].rearrange("p (s c) n -> p s c n", s=2)[:, :, :, :n],
                    CF[:, :, clo:clo + 2 * n].rearrange(
                        "p c (n s) -> p s c n", s=2))
                psa = pra_p.tile([128, 16, 64], F32, tag="rpsa", name="psa")
                psb = prb_p.tile([128, 8, 64], F32, tag="rpsb", name="psb")
                for m in range(16):
                    for k in range(8):
                        nc.tensor.matmul(
                            psa[:, m, :n],
                            wf_sb[:, (k * 24 + m) * 128:(k * 24 + m + 1) * 128],
                            ch[:, k, :n], start=(k == 0), stop=(k == 7))
                fwd_elem_a(lo, n, psa, Bt, g, 2)
                for m in range(16, 24):
                    for k in range(8):
                        nc.tensor.matmul(
                            psb[:, m - 16, :n],
                            wf_sb[:, (k * 24 + m) * 128:(k * 24 + m + 1) * 128],
                            ch[:, k, :n], start=(k == 0), stop=(k == 7))
                fwd_elem_b(lo, n, psb, g, t1)

            def fwd_fix511():
                # node-511 fix: slot 63 <- left child col 127 (masked). For
                # cores != 0 the mask zeroes the child, making this an
                # idempotent leaf recompute. Uses only the W_l half (k 0..3).
                Bt = sc_p.tile([128, 12, 65], F32, tag="B", name="Bx")
                ch = sc_p.tile([128, 8, 65], B16, tag="ch", name="chx")
                g = sc_p.tile([128, 24, 65], F32, tag="gates", name="gx")
                t1 = sc_p.tile([128, 4, 65], F32, tag="t1", name="t1x")
                nc.scalar.activation(ch[:, 0:4, 0:1], HF[:, :, 127:128],
                                     AF.Identity, scale=mask_sb[:, 0:1])
                nc.gpsimd.tensor_copy(Bt[:, 4:8, 0:1], CF[:, :, 127:128])
                nc.gpsimd.tensor_scalar_mul(Bt[:, 4:8, 0:1], Bt[:, 4:8, 0:1],
                                            mask_sb[:, 0:1])
                psa = pra_p.tile([128, 16, 64], F32, tag="rpsa", name="psax")
                psb = prb_p.tile([128, 8, 64], F32, tag="rpsb", name="psbx")
                for m in range(16):
                    for k in range(4):
                        nc.tensor.matmul(
                            psa[:, m, 0:1],
                            wf_sb[:, (k * 24 + m) * 128:(k * 24 + m + 1) * 128],
                            ch[:, k, 0:1], start=(k == 0), stop=(k == 3))
                fwd_elem_a(63, 1, psa, Bt, g, 1)
                for m in range(16, 24):
                    for k in range(4):
                        nc.tensor.matmul(
                            psb[:, m - 16, 0:1],
                            wf_sb[:, (k * 24 + m) * 128:(k * 24 + m + 1) * 128],
                            ch[:, k, 0:1], start=(k == 0), stop=(k == 3))
                fwd_elem_b(63, 1, psb, g, t1)

            def bwd_unit(lo, n, plo):
                Bt = sc_p.tile([128, 12, 65], F32, tag="B", name="Bb")
                ch = sc_p.tile([128, 8, 65], B16, tag="ch", name="chb")
                g = sc_p.tile([128, 24, 65], F32, tag="gates", name="gb")
                t1 = sc_p.tile([128, 4, 65], F32, tag="t1", name="t1b")
                if n == 1:
                    nc.scalar.activation(ch[:, 0:4, 0:1], HB[:, :, plo:plo + 1],
                                         AF.Identity)
                    nc.gpsimd.tensor_copy(Bt[:, 4:8, 0:1], CB[:, :, plo:plo + 1])
                else:
                    m2 = n // 2
                    src_h = HB[:, :, plo:plo + m2].unsqueeze(3).broadcast_to(
                        [128, 4, m2, 2])
                    src_c = CB[:, :, plo:plo + m2].unsqueeze(3).broadcast_to(
                        [128, 4, m2, 2])
                    nc.scalar.activation(
                        ch[:, 0:4, 0:n].rearrange("p c (a b) -> p c a b", b=2),
                        src_h, AF.Identity)
                    nc.gpsimd.tensor_copy(
                        Bt[:, 4:8, 0:n].rearrange("p c (a b) -> p c a b", b=2),
                        src_c)
                psa = pra_p.tile([128, 16, 64], F32, tag="rpsa", name="psba")
                psb = prb_p.tile([128, 8, 64], F32, tag="rpsb", name="psbb")
                for m in range(12):
                    for k in range(4):
                        nc.tensor.matmul(
                            psa[:, m, :n],
                            wb_sb[:, (k * 20 + m) * 128:(k * 20 + m + 1) * 128],
                            ch[:, k, :n], start=(k == 0), stop=(k == 3))
                bwd_elem_a(lo, n, psa, Bt, g)
                for m in range(12, 20):
                    for k in range(4):
                        nc.tensor.matmul(
                            psb[:, m - 12, :n],
                            wb_sb[:, (k * 20 + m) * 128:(k * 20 + m + 1) * 128],
                            ch[:, k, :n], start=(k == 0), stop=(k == 3))
                bwd_elem_b(lo, n, psb, g, t1)

            def bwd_root():
                Bt = sc_p.tile([128, 12, 65], F32, tag="B", name="Br")
                g = sc_p.tile([128, 24, 65], F32, tag="gates", name="gr")
                t1 = sc_p.tile([128, 4, 65], F32, tag="t1", name="t1r")
                bwd_elem_a(128, 1, None, Bt, g)
                bwd_elem_b(128, 1, None, g, t1)

            def bwd_sel():
                # copy own root (col 135+c) into local slot 0
                tmp = sc_p.tile([128, 4, 8], F32, tag="pseltmp", name="pseltmp")
                pb = psel_sb[:, :].unsqueeze(1).broadcast_to([128, 4, 8])
                nc.vector.tensor_mul(tmp[:], HB[:, :, 135:143], pb)
                nc.vector.reduce_sum(HB[:, :, 0], tmp[:], mybir.AxisListType.X)
                tmp2 = sc_p.tile([128, 4, 8], F32, tag="pseltmp", name="pseltmp2")
                nc.vector.tensor_mul(tmp2[:], CB[:, :, 135:143], pb)
                nc.vector.reduce_sum(CB[:, :, 0], tmp2[:], mybir.AxisListType.X)

            def pre_chunk(gidx, wpb):
                for mi in range(4):
                    m = gidx * 4 + mi
                    ps = pp_p.tile([128, 143], F32, tag="pps", name="pps")
                    for k in range(8):
                        nc.tensor.matmul(
                            ps[:],
                            wpb[:, (k * 4 + mi) * 128:(k * 4 + mi + 1) * 128],
                            ftile[:, k, :], start=(k == 0), stop=(k == 7))
                    if m < 28:
                        nc.scalar.activation(PRE_F[:, m, :], ps[:], AF.Identity,
                                             bias=bf_sb[:, m:m + 1])
                    else:
                        nc.scalar.activation(PRE_B[:, m - 28, :], ps[:],
                                             AF.Identity,
                                             bias=bb_sb[:, m - 28:m - 27])

            # ---------------- layer loop ----------------
            for l in range(L):
                # weight stream block (sync queue) in consumption order;
                # for l=1 these triggers sit behind l=0's and prefetch into
                # free slots during layer-0 compute.
                bf_sb = b_p.tile([128, 28], F32, tag="bf", name="bf_sb")
                bb_sb = b_p.tile([128, 24], F32, tag="bb", name="bb_sb")
                nc.scalar.dma_start(bf_sb[:], biasf_d[l][:])
                nc.scalar.dma_start(bb_sb[:], biasb_d[l][:])
                wpb_list = []
                for g in range(13):
                    wpb = ws_p.tile([128, 4096], B16, tag="wpre", name=f"wpb{g}")
                    nc.scalar.dma_start(wpb[:], wpre_d[l][g])
                    wpb_list.append(wpb)
                wf_sb = w_p.tile([128, 8 * 24 * 128], B16, tag="wf", name="wf_sb")
                wb_sb = w_p.tile([128, 4 * 20 * 128], B16, tag="wb", name="wb_sb")
                nc.scalar.dma_start(wf_sb[:], wrecf_d[l][:])
                nc.scalar.dma_start(wb_sb[:], wrecb_d[l][:])

                PRE_F = pre_p.tile([128, 28, NCOL], B16, tag="pref", name="PRE_F")
                PRE_B = pre_p.tile([128, 24, NCOL], B16, tag="preb", name="PRE_B")

                if l == 0:
                    ftile = pre_p.tile([128, 8, NCOL], B16, tag="ft", name="ftile")
                    nc.scalar.dma_start(
                        ftile[:].rearrange("p c n -> p (c n)"), featsT_d[:])

                # ---- pre-projections: PRE = W_pre @ feats (feature-major) ----
                for g in range(7):          # PRE_F chunks (m 0..27)
                    pre_chunk(g, wpb_list[g])
                Bleaf = sc_p.tile([128, 12, 65], F32, tag="B", name="Bleaf")
                gleaf = sc_p.tile([128, 24, 65], F32, tag="gates", name="gleaf")
                t1leaf = sc_p.tile([128, 4, 65], F32, tag="t1", name="t1leaf")
                fwd_elem_a(63, 65, None, Bleaf, gleaf, 0)
                fwd_elem_b(63, 65, None, gleaf, t1leaf)
                for g in range(7, 13):      # PRE_B chunks (m 28..51)
                    pre_chunk(g, wpb_list[g])
                bwd_root()
                fwd_fix511()

                # ---- interleaved recurrence ----
                fwd_unit(31, 32, 63)
                bwd_unit(129, 2, 128)
                fwd_unit(15, 16, 31)
                bwd_unit(131, 4, 129)
                fwd_unit(7, 8, 15)
                bwd_unit(135, 8, 131)
                fwd_unit(3, 4, 7)
                bwd_sel()
                bwd_unit(1, 2, 0)
                fwd_unit(1, 2, 3)
                bwd_unit(3, 4, 1)
                fwd_unit(0, 1, 1)

                # AllGather the 8 subtree roots' (h, c) — staged on gpsimd
                ccin = dram_p.tile([1024], F32, tag="ccin", name="ccin")
                ccout = dram_p.tile([8, 1024], F32, tag="ccout", name="ccout",
                                    addr_space="Shared")
                nc.gpsimd.dma_start(
                    ccin[0:512].rearrange("(c p) -> p c", c=4, p=128), HF[:, :, 0])
                nc.gpsimd.dma_start(
                    ccin[512:1024].rearrange("(c p) -> p c", c=4, p=128), CF[:, :, 0])
                nc.gpsimd.collective_compute(
                    "AllGather", mybir.AluOpType.bypass,
                    ins=[ccin.opt()], outs=[ccout.opt()],
                    replica_groups=[list(range(NCORES))])

                # bwd chain fills the AllGather window
                bwd_unit(7, 8, 3)
                bwd_unit(15, 16, 7)
                bwd_unit(31, 32, 15)
                bwd_unit(63, 64, 31)
                bwd_unit(127, 1, 63)    # node 1023

                if l + 1 < L:
                    # hb half of next-layer features is final here; copy it
                    # while the fwd top levels run (scalar queue)
                    ftile_n = pre_p.tile([128, 8, NCOL], B16, tag="ft",
                                         name="ftile1")
                    for k in range(4, 8):
                        nc.scalar.activation(ftile_n[:, k, :], HB[:, k % 4, :],
                                             AF.Identity)

                # AG consumers (gpsimd queue; they wait on the collective):
                # one 3D-AP DMA into a staging tile, then two vector copies
                ccv = ccout[:].rearrange("g (c p) -> p (g c)", c=8, p=128)
                stage = sc_p.tile([128, 64], F32, tag="agstage", name="agstage")
                nc.gpsimd.dma_start(stage[:], ccv)
                sgv = stage[:].rearrange("p (g c) -> p c g", g=8, c=8)
                nc.vector.tensor_copy(HF[:, :, 135:143], sgv[:, 0:4, :])
                nc.vector.tensor_copy(CF[:, :, 135:143], sgv[:, 4:8, :])

                # fwd top levels (consume the AllGather)
                fwd_unit(131, 4, 135)   # top level 2 (nodes 3..6)
                fwd_unit(129, 2, 131)   # top level 1
                fwd_unit(128, 1, 129)   # root

                if l + 1 < L:
                    for k in range(4):
                        nc.vector.tensor_copy(ftile_n[:, k, :], HF[:, k % 4, :])
                    ftile = ftile_n

            # ---- outputs ----
            olv = out_loc_d[:].rearrange("(c p) n -> p c n", c=8, p=128)
            nc.gpsimd.dma_start(olv[:, 0:4, :], HF[:, :, 0:128])
            nc.gpsimd.dma_start(olv[:, 4:8, :], HB[:, :, 0:128])
            otv = out_top_d[:].rearrange("(c p) n -> p c n", c=8, p=128)
            nc.gpsimd.dma_start(otv[:, 0:4, :], HF[:, :, 128:135])
            nc.gpsimd.dma_start(otv[:, 4:8, :], HB[:, :, 128:135])

    nc.finalize()
    return nc


_program_cache = None


def kernel(features, f_px_w, f_px_b, f_x_w, f_x_b, f_l_w, f_l_b, f_r_w, f_r_b,
           b_px_w, b_px_b, b_x_w, b_x_b, b_h_w, b_h_b, left, right, parent):
    global _program_cache, _last_results
    features = np.asarray(features, dtype=np.float32)
    as32 = lambda a: np.asarray(a, dtype=np.float32)

    # ---- host-side packing ----
    shared = {}
    for l in range(L):
        fx = _perm_rows(as32(f_x_w[l]), PERM_F)
        bx = _perm_rows(as32(b_x_w[l]), PERM_B)
        wpre = np.concatenate([fx, as32(f_px_w[l]), bx, as32(b_px_w[l])],
                              axis=0)                      # [6656, 1024]
        t = wpre.reshape(13, 4, 128, 8, 128).transpose(0, 4, 3, 1, 2)
        shared[f"wpre{l}"] = np.ascontiguousarray(
            t.reshape(13, 128, 4096).astype(BF16))

        wrf = np.concatenate([_perm_rows(as32(f_l_w[l]), PERM_F),
                              _perm_rows(as32(f_r_w[l]), PERM_F)],
                             axis=1)                       # [3072, 1024]
        t = wrf.reshape(24, 128, 8, 128).transpose(3, 2, 0, 1)
        shared[f"wrecf{l}"] = np.ascontiguousarray(
            t.reshape(128, 8 * 24 * 128).astype(BF16))

        wrb = _perm_rows(as32(b_h_w[l]), PERM_B)           # [2560, 512]
        t = wrb.reshape(20, 128, 4, 128).transpose(3, 2, 0, 1)
        shared[f"wrecb{l}"] = np.ascontiguousarray(
            t.reshape(128, 4 * 20 * 128).astype(BF16))

        bf = np.concatenate([
            _perm_rows(as32(f_x_b[l]) + as32(f_l_b[l]) + as32(f_r_b[l]), PERM_F),
            as32(f_px_b[l])])                              # [3584]
        shared[f"biasf{l}"] = np.ascontiguousarray(bf.reshape(28, 128).T)
        bb = np.concatenate([
            _perm_rows(as32(b_x_b[l]) + as32(b_h_b[l]), PERM_B),
            as32(b_px_b[l])])
        shared[f"biasb{l}"] = np.ascontiguousarray(bb.reshape(24, 128).T)

    in_maps = []
    ids_all = []
    for c in range(NCORES):
        ids = _node_ids(c)
        ids_all.append(ids)
        ft = features[ids]                                 # [143, 1024]
        ftT = ft.T.reshape(8, 128, NCOL).transpose(1, 0, 2)  # [128, 8, 143]
        m = {k: v for k, v in shared.items()}
        m["featsT"] = np.ascontiguousarray(
            ftT.reshape(128, 8 * NCOL).astype(BF16))
        m["mask"] = np.full((128, 1), 1.0 if c == 0 else 0.0, np.float32)
        ps = np.zeros((128, 8), np.float32)
        ps[:, c] = 1.0
        m["psel"] = ps
        in_maps.append(m)

    if _program_cache is None:
        _program_cache = _build_program()
    nc = _program_cache

    trace = bool(os.environ.get("KERNEL_TRACE"))
    tdir = os.environ.get("KERNEL_TRACE_DIR") or None
    res = run_bass_kernel_spmd(nc, in_maps, core_ids=list(range(NCORES)),
                               trace=trace, tmpdir=tdir)
    _last_results = res

    out = np.empty((N, 2 * H), np.float32)
    for c in range(NCORES):
        loc = res.results[c]["out_loc"]                    # [1024, 128]
        nloc = 128 if c == 0 else 127
        out[ids_all[c][0:nloc]] = loc[:, 0:nloc].T
    out[0:7] = res.results[0]["out_top"].T
    return out


# revision 16
# speedup vs baseline: 1.4075x; 1.0397x over previous
"""Multi-layer bidirectional Tree-LSTM on 8 TRN2 NeuronCores.

Strategy: the input is a complete binary tree of 1024 nodes. Below level 3
there are 8 independent subtrees (rooted at nodes 7..14) -> one subtree per
core (data parallel). The top 7 nodes (0..6) are computed replicated on all
cores; one small AllGather per layer exchanges the 8 subtree-root (h, c)
pairs for the leaves->root direction.

On-device layout is feature-major (hidden dim on partitions, nodes on the
free axis), weights are stationary (bf16, FWL) and node columns stream.

Perf structure (v2):
  - weights host-packed to [128, X] contiguous DRAM layouts (one fat
    descriptor per partition), streamed on the Sync queue in consumption
    order; layer-1 weights prefetch during layer 0 (wpre pool 8-deep).
  - fwd and bwd recurrence chains interleaved per level so each level's
    elementwise tail hides under the other chain's matmuls (keeps PE busy,
    HAM clock warm).
  - child gathers on GpSimd; collectives + cc staging DMAs + outputs on
    GpSimd's queue; elementwise on Vector; activations on Scalar.
  - gate blocks reordered host-side to [ig,fl,fr,og,r,u] (fwd) /
    [ig,f,og,r,u] (bwd) so c = sum of products is one fused mul + adds
    and one sigmoid call covers all sigmoid gates.

Per-core column layout (143 columns):
  0..126   : BFS slots of subtree(7+c)  (slot s, level k=floor(log2(s+1)))
  127      : node 1023 (replicated on every core; only core 0's is used)
  128..134 : top nodes 0..6 (replicated)
  135..142 : subtree roots 7..14 (fwd: from AllGather; bwd: replicated)
"""

import os
import sys

for _p in ("/opt/trn_rl_repo",):
    if _p not in sys.path and os.path.isdir(_p):
        sys.path.insert(0, _p)

import numpy as np
import ml_dtypes

try:
    import jax
    jax.config.update("jax_compilation_cache_dir", os.environ.get("KERNEL_JAX_CACHE", "/tmp/jax_neff_cache"))
    jax.config.update("jax_persistent_cache_min_compile_time_secs", 5.0)
    jax.config.update("jax_persistent_cache_min_entry_size_bytes", 0)
except Exception:
    pass

import concourse.bass as bass
import concourse.mybir as mybir
from concourse import bacc
from concourse.tile import TileContext
from concourse.bass_utils import run_bass_kernel_spmd

BF16 = ml_dtypes.bfloat16
F32 = mybir.dt.float32
B16 = mybir.dt.bfloat16
AF = mybir.ActivationFunctionType

N, D, H, L = 1024, 1024, 512, 2
NCOL = 143  # 127 subtree + node1023 + 7 top + 8 roots
NCORES = 8

# gate block permutations (blocks of H rows)
# fwd reference order: ig, og, fl, fr, u, r  ->  new: ig, fl, fr, u, og, r
# (c-path gates first so the first PSUM group [ig,fl,fr,u] unblocks the
#  c computation while the [og,r] group is still in the matmul queue)
PERM_F = [0, 2, 3, 4, 1, 5]
# bwd reference order: ig, og, f, u, r       ->  new: ig, f, u, og, r
PERM_B = [0, 2, 3, 1, 4]

_last_results = None  # stashed BassKernelResults for test.py


def _node_ids(c):
    ids = []
    for k in range(7):
        base = (8 + c) * (1 << k) - 1
        ids.extend(range(base, base + (1 << k)))
    ids.append(1023)
    ids.extend(range(0, 7))
    ids.extend(range(7, 15))
    return np.asarray(ids, dtype=np.int64)


def _perm_rows(a, perm):
    return np.concatenate([a[i * H:(i + 1) * H] for i in perm], axis=0)


def _build_program():
    nc = bacc.Bacc("TRN2", target_bir_lowering=False, debug=False,
                   num_devices=NCORES)

    featsT_d = nc.dram_tensor("featsT", [128, 8 * NCOL], B16, kind="ExternalInput")
    wpre_d, wrecf_d, wrecb_d, biasf_d, biasb_d = [], [], [], [], []
    for l in range(L):
        wpre_d.append(nc.dram_tensor(f"wpre{l}", [13, 128, 4096], B16,
                                     kind="ExternalInput"))
        wrecf_d.append(nc.dram_tensor(f"wrecf{l}", [128, 8 * 24 * 128], B16,
                                      kind="ExternalInput"))
        wrecb_d.append(nc.dram_tensor(f"wrecb{l}", [128, 4 * 20 * 128], B16,
                                      kind="ExternalInput"))
        biasf_d.append(nc.dram_tensor(f"biasf{l}", [128, 28], F32,
                                      kind="ExternalInput"))
        biasb_d.append(nc.dram_tensor(f"biasb{l}", [128, 24], F32,
                                      kind="ExternalInput"))
    mask_d = nc.dram_tensor("mask", [128, 1], F32, kind="ExternalInput")
    psel_d = nc.dram_tensor("psel", [128, 8], F32, kind="ExternalInput")
    out_loc_d = nc.dram_tensor("out_loc", [1024, 128], F32, kind="ExternalOutput")
    out_top_d = nc.dram_tensor("out_top", [1024, 7], F32, kind="ExternalOutput")

    with TileContext(nc) as tc:
        with (
            tc.tile_pool(name="state", bufs=1) as state_p,
            tc.tile_pool(name="weights", bufs=1) as w_p,
            tc.tile_pool(name="bias", bufs=2) as b_p,
            tc.tile_pool(name="pre", bufs=1) as pre_p,
            tc.tile_pool(name="wstream", bufs=8) as ws_p,
            tc.tile_pool(name="scratch", bufs=3) as sc_p,
            tc.tile_pool(name="psum_pre", bufs=2, space="PSUM") as pp_p,
            tc.tile_pool(name="psum_reca", bufs=2, space="PSUM") as pra_p,
            tc.tile_pool(name="psum_recb", bufs=2, space="PSUM") as prb_p,
            tc.tile_pool(name="dram", bufs=1, space="DRAM") as dram_p,
        ):
            HF = state_p.tile([128, 4, NCOL], F32, name="HF")
            CF = state_p.tile([128, 4, NCOL], F32, name="CF")
            HB = state_p.tile([128, 4, NCOL], F32, name="HB")
            CB = state_p.tile([128, 4, NCOL], F32, name="CB")
            mask_sb = state_p.tile([128, 1], F32, name="mask_sb")
            psel_sb = state_p.tile([128, 8], F32, name="psel_sb")
            nc.scalar.dma_start(mask_sb[:], mask_d[:])
            nc.scalar.dma_start(psel_sb[:], psel_d[:])

            # ---------------- elementwise helpers ----------------
            # fwd gate chunks: 0:4 ig | 4:8 fl | 8:12 fr | 12:16 u
            #                  | 16:20 og | 20:24 r ; PRE_F 24:28 px
            # psum group a = chunks 0..15 (c path), group b = 16..23 (h path)
            def fwd_elem_a(lo, n, psa, Bt, g, nch):
                cn = CF[:, :, lo:lo + n]
                if psa is None:
                    nc.scalar.activation(g[:, 0:12, :n], PRE_F[:, 0:12, lo:lo + n],
                                         AF.Sigmoid)
                    nc.scalar.activation(Bt[:, 0:4, :n], PRE_F[:, 12:16, lo:lo + n],
                                         AF.Tanh)
                else:
                    nc.vector.tensor_add(g[:, 0:16, :n], psa[:, 0:16, :n],
                                         PRE_F[:, 0:16, lo:lo + n])
                    nc.scalar.activation(g[:, 0:12, :n], g[:, 0:12, :n], AF.Sigmoid)
                    nc.scalar.activation(Bt[:, 0:4, :n], g[:, 12:16, :n], AF.Tanh)
                if nch == 2:
                    nc.vector.tensor_mul(g[:, 0:12, :n], g[:, 0:12, :n],
                                         Bt[:, 0:12, :n])
                    nc.vector.tensor_add(cn, g[:, 0:4, :n], g[:, 4:8, :n])
                    nc.vector.tensor_add(cn, cn, g[:, 8:12, :n])
                elif nch == 1:
                    nc.vector.tensor_mul(g[:, 0:8, :n], g[:, 0:8, :n],
                                         Bt[:, 0:8, :n])
                    nc.vector.tensor_add(cn, g[:, 0:4, :n], g[:, 4:8, :n])
                else:
                    nc.vector.tensor_mul(cn, g[:, 0:4, :n], Bt[:, 0:4, :n])

            def fwd_elem_b(lo, n, psb, g, t1):
                cn = CF[:, :, lo:lo + n]
                nc.scalar.activation(t1[:, :, :n], cn, AF.Tanh)
                if psb is None:
                    nc.scalar.activation(g[:, 16:24, :n], PRE_F[:, 16:24, lo:lo + n],
                                         AF.Sigmoid)
                else:
                    nc.vector.tensor_add(g[:, 16:24, :n], psb[:, 0:8, :n],
                                         PRE_F[:, 16:24, lo:lo + n])
                    nc.scalar.activation(g[:, 16:24, :n], g[:, 16:24, :n],
                                         AF.Sigmoid)
                t2 = sc_p.tile([128, 4, 65], F32, tag="t2", name="t2")
                nc.vector.tensor_mul(t2[:, :, :n], g[:, 16:20, :n], t1[:, :, :n])
                px = PRE_F[:, 24:28, lo:lo + n]
                nc.vector.tensor_sub(t2[:, :, :n], t2[:, :, :n], px)
                nc.vector.tensor_mul(t2[:, :, :n], g[:, 20:24, :n], t2[:, :, :n])
                nc.vector.tensor_add(HF[:, :, lo:lo + n], t2[:, :, :n], px)

            # bwd gate chunks: 0:4 ig | 4:8 f | 8:12 u | 12:16 og | 16:20 r
            # PRE_B 20:24 px ; psum group a = 0..11, group b = 12..19
            def bwd_elem_a(lo, n, psa, Bt, g):
                cn = CB[:, :, lo:lo + n]
                if psa is None:
                    nc.scalar.activation(g[:, 0:8, :n], PRE_B[:, 0:8, lo:lo + n],
                                         AF.Sigmoid)
                    nc.scalar.activation(Bt[:, 0:4, :n], PRE_B[:, 8:12, lo:lo + n],
                                         AF.Tanh)
                    nc.vector.tensor_mul(cn, g[:, 0:4, :n], Bt[:, 0:4, :n])
                else:
                    nc.vector.tensor_add(g[:, 0:12, :n], psa[:, 0:12, :n],
                                         PRE_B[:, 0:12, lo:lo + n])
                    nc.scalar.activation(g[:, 0:8, :n], g[:, 0:8, :n], AF.Sigmoid)
                    nc.scalar.activation(Bt[:, 0:4, :n], g[:, 8:12, :n], AF.Tanh)
                    nc.vector.tensor_mul(g[:, 0:8, :n], g[:, 0:8, :n],
                                         Bt[:, 0:8, :n])
                    nc.vector.tensor_add(cn, g[:, 0:4, :n], g[:, 4:8, :n])

            def bwd_elem_b(lo, n, psb, g, t1):
                cn = CB[:, :, lo:lo + n]
                nc.scalar.activation(t1[:, :, :n], cn, AF.Tanh)
                if psb is None:
                    nc.scalar.activation(g[:, 12:20, :n], PRE_B[:, 12:20, lo:lo + n],
                                         AF.Sigmoid)
                else:
                    nc.vector.tensor_add(g[:, 12:20, :n], psb[:, 0:8, :n],
                                         PRE_B[:, 12:20, lo:lo + n])
                    nc.scalar.activation(g[:, 12:20, :n], g[:, 12:20, :n],
                                         AF.Sigmoid)
                t2 = sc_p.tile([128, 4, 65], F32, tag="t2", name="t2b")
                nc.vector.tensor_mul(t2[:, :, :n], g[:, 12:16, :n], t1[:, :, :n])
                px = PRE_B[:, 20:24, lo:lo + n]
                nc.vector.tensor_sub(t2[:, :, :n], t2[:, :, :n], px)
                nc.vector.tensor_mul(t2[:, :, :n], g[:, 16:20, :n], t2[:, :, :n])
                nc.vector.tensor_add(HB[:, :, lo:lo + n], t2[:, :, :n], px)

            # ---------------- recurrence units ----------------
            def fwd_unit(lo, n, clo):
                Bt = sc_p.tile([128, 12, 65], F32, tag="B", name="Bf")
                ch = sc_p.tile([128, 8, 65], B16, tag="ch", name="ch")
                g = sc_p.tile([128, 24, 65], F32, tag="gates", name="g")
                t1 = sc_p.tile([128, 4, 65], F32, tag="t1", name="t1")
                # child h gather (MM-critical) on Scalar, one fused op
                nc.scalar.activation(
                    ch[:].rearrange("p (s c) n -> p s c n", s=2)[:, :, :, :n],
                    HF[:, :, clo:clo + 2 * n].rearrange(
                        "p c (n s) -> p s c n", s=2),
                    AF.Identity)
                # child c gather (elem-critical) on GpSimd
                nc.gpsimd.tensor_copy(
                    Bt[:, 4:12, :].rearrange("p (s c) n -> p s c n", s=2)[:, :, :, :n],
                    CF[:, :, clo:clo + 2 * n].rearrange(
                        "p c (n s) -> p s c n", s=2))
                psa = pra_p.tile([128, 16, 64], F32, tag="rpsa", name="psa")
                psb = prb_p.tile([128, 8, 64], F32, tag="rpsb", name="psb")
                for m in range(16):
                    for k in range(8):
                        nc.tensor.matmul(
                            psa[:, m, :n],
                            wf_sb[:, (k * 24 + m) * 128:(k * 24 + m + 1) * 128],
                            ch[:, k, :n], start=(k == 0), stop=(k == 7))
                fwd_elem_a(lo, n, psa, Bt, g, 2)
                for m in range(16, 24):
                    for k in range(8):
                        nc.tensor.matmul(
                            psb[:, m - 16, :n],
                            wf_sb[:, (k * 24 + m) * 128:(k * 24 + m + 1) * 128],
                            ch[:, k, :n], start=(k == 0), stop=(k == 7))
                fwd_elem_b(lo, n, psb, g, t1)

            def fwd_fix511():
                # node-511 fix: slot 63 <- left child col 127 (masked). For
                # cores != 0 the mask zeroes the child, making this an
                # idempotent leaf recompute. Uses only the W_l half (k 0..3).
                Bt = sc_p.tile([128, 12, 65], F32, tag="B", name="Bx")
                ch = sc_p.tile([128, 8, 65], B16, tag="ch", name="chx")
                g = sc_p.tile([128, 24, 65], F32, tag="gates", name="gx")
                t1 = sc_p.tile([128, 4, 65], F32, tag="t1", name="t1x")
                nc.scalar.activation(ch[:, 0:4, 0:1], HF[:, :, 127:128],
                                     AF.Identity, scale=mask_sb[:, 0:1])
                nc.gpsimd.tensor_copy(Bt[:, 4:8, 0:1], CF[:, :, 127:128])
                nc.gpsimd.tensor_scalar_mul(Bt[:, 4:8, 0:1], Bt[:, 4:8, 0:1],
                                            mask_sb[:, 0:1])
                psa = pra_p.tile([128, 16, 64], F32, tag="rpsa", name="psax")
                psb = prb_p.tile([128, 8, 64], F32, tag="rpsb", name="psbx")
                for m in range(16):
                    for k in range(4):
                        nc.tensor.matmul(
                            psa[:, m, 0:1],
                            wf_sb[:, (k * 24 + m) * 128:(k * 24 + m + 1) * 128],
                            ch[:, k, 0:1], start=(k == 0), stop=(k == 3))
                fwd_elem_a(63, 1, psa, Bt, g, 1)
                for m in range(16, 24):
                    for k in range(4):
                        nc.tensor.matmul(
                            psb[:, m - 16, 0:1],
                            wf_sb[:, (k * 24 + m) * 128:(k * 24 + m + 1) * 128],
                            ch[:, k, 0:1], start=(k == 0), stop=(k == 3))
                fwd_elem_b(63, 1, psb, g, t1)

            def bwd_unit(lo, n, plo):
                Bt = sc_p.tile([128, 12, 65], F32, tag="B", name="Bb")
                ch = sc_p.tile([128, 8, 65], B16, tag="ch", name="chb")
                g = sc_p.tile([128, 24, 65], F32, tag="gates", name="gb")
                t1 = sc_p.tile([128, 4, 65], F32, tag="t1", name="t1b")
                if n == 1:
                    nc.scalar.activation(ch[:, 0:4, 0:1], HB[:, :, plo:plo + 1],
                                         AF.Identity)
                    nc.gpsimd.tensor_copy(Bt[:, 4:8, 0:1], CB[:, :, plo:plo + 1])
                else:
                    m2 = n // 2
                    src_h = HB[:, :, plo:plo + m2].unsqueeze(3).broadcast_to(
                        [128, 4, m2, 2])
                    src_c = CB[:, :, plo:plo + m2].unsqueeze(3).broadcast_to(
                        [128, 4, m2, 2])
                    nc.scalar.activation(
                        ch[:, 0:4, 0:n].rearrange("p c (a b) -> p c a b", b=2),
                        src_h, AF.Identity)
                    nc.gpsimd.tensor_copy(
                        Bt[:, 4:8, 0:n].rearrange("p c (a b) -> p c a b", b=2),
                        src_c)
                psa = pra_p.tile([128, 16, 64], F32, tag="rpsa", name="psba")
                psb = prb_p.tile([128, 8, 64], F32, tag="rpsb", name="psbb")
                for m in range(12):
                    for k in range(4):
                        nc.tensor.matmul(
                            psa[:, m, :n],
                            wb_sb[:, (k * 20 + m) * 128:(k * 20 + m + 1) * 128],
                            ch[:, k, :n], start=(k == 0), stop=(k == 3))
                bwd_elem_a(lo, n, psa, Bt, g)
                for m in range(12, 20):
                    for k in range(4):
                        nc.tensor.matmul(
                            psb[:, m - 12, :n],
                            wb_sb[:, (k * 20 + m) * 128:(k * 20 + m + 1) * 128],
                            ch[:, k, :n], start=(k == 0), stop=(k == 3))
                bwd_elem_b(lo, n, psb, g, t1)

            def bwd_root():
                Bt = sc_p.tile([128, 12, 65], F32, tag="B", name="Br")
                g = sc_p.tile([128, 24, 65], F32, tag="gates", name="gr")
                t1 = sc_p.tile([128, 4, 65], F32, tag="t1", name="t1r")
                bwd_elem_a(128, 1, None, Bt, g)
                bwd_elem_b(128, 1, None, g, t1)

            def bwd_sel():
                # copy own root (col 135+c) into local slot 0
                tmp = sc_p.tile([128, 4, 8], F32, tag="pseltmp", name="pseltmp")
                pb = psel_sb[:, :].unsqueeze(1).broadcast_to([128, 4, 8])
                nc.vector.tensor_mul(tmp[:], HB[:, :, 135:143], pb)
                nc.vector.reduce_sum(HB[:, :, 0], tmp[:], mybir.AxisListType.X)
                tmp2 = sc_p.tile([128, 4, 8], F32, tag="pseltmp", name="pseltmp2")
                nc.vector.tensor_mul(tmp2[:], CB[:, :, 135:143], pb)
                nc.vector.reduce_sum(CB[:, :, 0], tmp2[:], mybir.AxisListType.X)

            def pre_chunk(gidx, wpb):
                for mi in range(4):
                    m = gidx * 4 + mi
                    ps = pp_p.tile([128, 143], F32, tag="pps", name="pps")
                    for k in range(8):
                        nc.tensor.matmul(
                            ps[:],
                            wpb[:, (k * 4 + mi) * 128:(k * 4 + mi + 1) * 128],
                            ftile[:, k, :], start=(k == 0), stop=(k == 7))
                    if m < 28:
                        nc.scalar.activation(PRE_F[:, m, :], ps[:], AF.Identity,
                                             bias=bf_sb[:, m:m + 1])
                    else:
                        nc.scalar.activation(PRE_B[:, m - 28, :], ps[:],
                                             AF.Identity,
                                             bias=bb_sb[:, m - 28:m - 27])

            # ---------------- layer loop ----------------
            # features first: its transfer must not queue behind the bulk
            # weights on the DMA ring (the first pre matmul waits on it)
            ftile = pre_p.tile([128, 8, NCOL], B16, tag="ft", name="ftile")
            nc.scalar.dma_start(ftile[:].rearrange("p c n -> p (c n)"),
                                featsT_d[:])
            for l in range(L):
                # weight stream block (sync queue) in consumption order;
                # for l=1 these triggers sit behind l=0's and prefetch into
                # free slots during layer-0 compute.
                bf_sb = b_p.tile([128, 28], F32, tag="bf", name="bf_sb")
                bb_sb = b_p.tile([128, 24], F32, tag="bb", name="bb_sb")
                nc.scalar.dma_start(bf_sb[:], biasf_d[l][:])
                nc.scalar.dma_start(bb_sb[:], biasb_d[l][:])
                wpb_list = []
                for g in range(13):
                    wpb = ws_p.tile([128, 4096], B16, tag="wpre", name=f"wpb{g}")
                    nc.scalar.dma_start(wpb[:], wpre_d[l][g])
                    wpb_list.append(wpb)
                wf_sb = w_p.tile([128, 8 * 24 * 128], B16, tag="wf", name="wf_sb")
                wb_sb = w_p.tile([128, 4 * 20 * 128], B16, tag="wb", name="wb_sb")
                nc.scalar.dma_start(wf_sb[:], wrecf_d[l][:])
                nc.scalar.dma_start(wb_sb[:], wrecb_d[l][:])

                PRE_F = pre_p.tile([128, 28, NCOL], B16, tag="pref", name="PRE_F")
                PRE_B = pre_p.tile([128, 24, NCOL], B16, tag="preb", name="PRE_B")

                # ---- pre-projections: PRE = W_pre @ feats (feature-major) ----
                for g in range(7):          # PRE_F chunks (m 0..27)
                    pre_chunk(g, wpb_list[g])
                Bleaf = sc_p.tile([128, 12, 65], F32, tag="B", name="Bleaf")
                gleaf = sc_p.tile([128, 24, 65], F32, tag="gates", name="gleaf")
                t1leaf = sc_p.tile([128, 4, 65], F32, tag="t1", name="t1leaf")
                fwd_elem_a(63, 65, None, Bleaf, gleaf, 0)
                fwd_elem_b(63, 65, None, gleaf, t1leaf)
                for g in range(7, 13):      # PRE_B chunks (m 28..51)
                    pre_chunk(g, wpb_list[g])
                bwd_root()
                fwd_fix511()

                # ---- interleaved recurrence ----
                fwd_unit(31, 32, 63)
                bwd_unit(129, 2, 128)
                fwd_unit(15, 16, 31)
                bwd_unit(131, 4, 129)
                fwd_unit(7, 8, 15)
                bwd_unit(135, 8, 131)
                fwd_unit(3, 4, 7)
                bwd_sel()
                bwd_unit(1, 2, 0)
                fwd_unit(1, 2, 3)
                bwd_unit(3, 4, 1)
                fwd_unit(0, 1, 1)

                # AllGather the 8 subtree roots' (h, c) — staging DMAs on the
                # otherwise-idle Sync ring so they never queue behind the
                # weight prefetch stream
                ccin = dram_p.tile([1024], F32, tag="ccin", name="ccin")
                ccout = dram_p.tile([8, 1024], F32, tag="ccout", name="ccout",
                                    addr_space="Shared")
                nc.sync.dma_start(
                    ccin[0:512].rearrange("(c p) -> p c", c=4, p=128), HF[:, :, 0])
                nc.sync.dma_start(
                    ccin[512:1024].rearrange("(c p) -> p c", c=4, p=128), CF[:, :, 0])
                nc.gpsimd.collective_compute(
                    "AllGather", mybir.AluOpType.bypass,
                    ins=[ccin.opt()], outs=[ccout.opt()],
                    replica_groups=[list(range(NCORES))])

                # bwd chain fills the AllGather window
                bwd_unit(7, 8, 3)
                bwd_unit(15, 16, 7)
                bwd_unit(31, 32, 15)
                bwd_unit(63, 64, 31)
                bwd_unit(127, 1, 63)    # node 1023

                if l + 1 < L:
                    # hb half of next-layer features is final here; copy it
                    # while the fwd top levels run (scalar queue)
                    ftile_n = pre_p.tile([128, 8, NCOL], B16, tag="ft",
                                         name="ftile1")
                    for k in range(4, 8):
                        nc.scalar.activation(ftile_n[:, k, :], HB[:, k % 4, :],
                                             AF.Identity)

                # AG consumers (gpsimd queue; they wait on the collective):
                # one 3D-AP DMA into a staging tile, then two vector copies
                ccv = ccout[:].rearrange("g (c p) -> p (g c)", c=8, p=128)
                stage = sc_p.tile([128, 64], F32, tag="agstage", name="agstage")
                nc.sync.dma_start(stage[:], ccv)
                sgv = stage[:].rearrange("p (g c) -> p c g", g=8, c=8)
                nc.gpsimd.tensor_copy(HF[:, :, 135:143], sgv[:, 0:4, :])
                nc.gpsimd.tensor_copy(CF[:, :, 135:143], sgv[:, 4:8, :])

                # fwd top levels (consume the AllGather)
                fwd_unit(131, 4, 135)   # top level 2 (nodes 3..6)
                fwd_unit(129, 2, 131)   # top level 1
                fwd_unit(128, 1, 129)   # root

                if l + 1 < L:
                    for k in range(4):
                        nc.vector.tensor_copy(ftile_n[:, k, :], HF[:, k % 4, :])
                    ftile = ftile_n

            # ---- outputs ----
            olv = out_loc_d[:].rearrange("(c p) n -> p c n", c=8, p=128)
            nc.gpsimd.dma_start(olv[:, 0:4, :], HF[:, :, 0:128])
            nc.gpsimd.dma_start(olv[:, 4:8, :], HB[:, :, 0:128])
            otv = out_top_d[:].rearrange("(c p) n -> p c n", c=8, p=128)
            nc.gpsimd.dma_start(otv[:, 0:4, :], HF[:, :, 128:135])
            nc.gpsimd.dma_start(otv[:, 4:8, :], HB[:, :, 128:135])

    nc.finalize()
    return nc


_program_cache = None


def kernel(features, f_px_w, f_px_b, f_x_w, f_x_b, f_l_w, f_l_b, f_r_w, f_r_b,
           b_px_w, b_px_b, b_x_w, b_x_b, b_h_w, b_h_b, left, right, parent):
    global _program_cache, _last_results
    features = np.asarray(features, dtype=np.float32)
    as32 = lambda a: np.asarray(a, dtype=np.float32)

    # ---- host-side packing ----
    shared = {}
    for l in range(L):
        fx = _perm_rows(as32(f_x_w[l]), PERM_F)
        bx = _perm_rows(as32(b_x_w[l]), PERM_B)
        wpre = np.concatenate([fx, as32(f_px_w[l]), bx, as32(b_px_w[l])],
                              axis=0)                      # [6656, 1024]
        t = wpre.reshape(13, 4, 128, 8, 128).transpose(0, 4, 3, 1, 2)
        shared[f"wpre{l}"] = np.ascontiguousarray(
            t.reshape(13, 128, 4096).astype(BF16))

        wrf = np.concatenate([_perm_rows(as32(f_l_w[l]), PERM_F),
                              _perm_rows(as32(f_r_w[l]), PERM_F)],
                             axis=1)                       # [3072, 1024]
        t = wrf.reshape(24, 128, 8, 128).transpose(3, 2, 0, 1)
        shared[f"wrecf{l}"] = np.ascontiguousarray(
            t.reshape(128, 8 * 24 * 128).astype(BF16))

        wrb = _perm_rows(as32(b_h_w[l]), PERM_B)           # [2560, 512]
        t = wrb.reshape(20, 128, 4, 128).transpose(3, 2, 0, 1)
        shared[f"wrecb{l}"] = np.ascontiguousarray(
            t.reshape(128, 4 * 20 * 128).astype(BF16))

        bf = np.concatenate([
            _perm_rows(as32(f_x_b[l]) + as32(f_l_b[l]) + as32(f_r_b[l]), PERM_F),
            as32(f_px_b[l])])                              # [3584]
        shared[f"biasf{l}"] = np.ascontiguousarray(bf.reshape(28, 128).T)
        bb = np.concatenate([
            _perm_rows(as32(b_x_b[l]) + as32(b_h_b[l]), PERM_B),
            as32(b_px_b[l])])
        shared[f"biasb{l}"] = np.ascontiguousarray(bb.reshape(24, 128).T)

    in_maps = []
    ids_all = []
    for c in range(NCORES):
        ids = _node_ids(c)
        ids_all.append(ids)
        ft = features[ids]                                 # [143, 1024]
        ftT = ft.T.reshape(8, 128, NCOL).transpose(1, 0, 2)  # [128, 8, 143]
        m = {k: v for k, v in shared.items()}
        m["featsT"] = np.ascontiguousarray(
            ftT.reshape(128, 8 * NCOL).astype(BF16))
        m["mask"] = np.full((128, 1), 1.0 if c == 0 else 0.0, np.float32)
        ps = np.zeros((128, 8), np.float32)
        ps[:, c] = 1.0
        m["psel"] = ps
        in_maps.append(m)

    if _program_cache is None:
        _program_cache = _build_program()
    nc = _program_cache

    trace = bool(os.environ.get("KERNEL_TRACE"))
    tdir = os.environ.get("KERNEL_TRACE_DIR") or None
    res = run_bass_kernel_spmd(nc, in_maps, core_ids=list(range(NCORES)),
                               trace=trace, tmpdir=tdir)
    _last_results = res

    out = np.empty((N, 2 * H), np.float32)
    for c in range(NCORES):
        loc = res.results[c]["out_loc"]                    # [1024, 128]
        nloc = 128 if c == 0 else 127
        out[ids_all[c][0:nloc]] = loc[:, 0:nloc].T
    out[0:7] = res.results[0]["out_top"].T
    return out
